# revision 1
# baseline (speedup 1.0000x reference)
"""Trainium2 Bass kernel for the AdSBHNet holographic-potential problem.

Computes, for a batch of turning points zs (B=8192) and small polynomial
coefficient vectors a, b plus scalars logcoef/shift:

    L  = 4 zs/pi * trapz-integral over y of  sqrt(fs) * W2 * y * sqrt(gn/(gd*t1))
    Vc = 4 pi/zs * trapz of (y/W2) * S * (fz - sqrt(t1*fz))
    Vd = 2 pi (1-zs) * trapz of sqrt(fzd*gnd/(gdd*zd^4))
    out = [L, exp(logcoef)*(Vc - Vd) + shift]

Sharding: data-parallel over zs across 8 NeuronCores (1024 each). On each
core the 1000 quadrature points sit on SBUF partitions (8 tiles x 125) and
the 1024 zs values on the free dimension. All bilinear "polynomial" grids
(fz, gn, gd, t1, gdd, fzd, gnd) are low-rank in (y-coeffs x zs-power-rows)
and are built by TensorEngine matmuls straight into PSUM; DVE/ACT/GPSIMD do
the sqrt/div chain; the quadrature reductions are K=125 matmuls with the
trapezoid weights as the stationary operand, accumulated across y-tiles in
PSUM. Everything f32 (f32r on the PE).
"""

import math
import numpy as np

B_TOTAL = 8192
NCORES = 8
BPC = B_TOTAL // NCORES          # 1024 zs per core
S = 1000                         # quadrature steps
NT = 8                           # y tiles per core
P = S // NT                      # 125 partitions per y tile
HALF = 512                       # matmul moving free dim

_COMPILED = {}


def _trapz_weights(x, append_one):
    """Node weights on the raw integrand I_0..I_{S-1} reproducing the
    reference's trapz over [0, x..., (1)] with linear extrapolation to 0
    (and a zero appended at 1 when append_one)."""
    n = len(x)
    u = np.zeros(n)
    u[0] = 0.5 * (x[1] - 0.0)
    u[1:-1] = 0.5 * (x[2:] - x[:-2])
    if append_one:
        u[-1] = 0.5 * (1.0 - x[-2])
    else:
        u[-1] = 0.5 * (x[-1] - x[-2])
    w_i0 = 0.5 * x[0]
    d = x[1] - x[0]
    u[0] += w_i0 * (1.0 + x[0] / d)
    u[1] += w_i0 * (-x[0] / d)
    return u


def _build_host_tables(a, b, logcoef, shift, zs):
    """All small derived constants, in float64, cast to f32 at the end."""
    a = np.asarray(a, np.float64)
    b = np.asarray(b, np.float64)
    lc = float(np.asarray(logcoef).reshape(-1)[0])
    sh = float(np.asarray(shift).reshape(-1)[0])
    zs = np.asarray(zs, np.float64)

    y = np.linspace(0.001, 0.999, S)
    y2 = np.linspace(0.001, 1.0, S)

    fa1 = 4.0 / 3.0 * a[0]
    fa2 = 2.0 * a[1]
    fa4 = -(1.0 + fa1 + fa2)

    w = 1.0 - y * y
    W2 = w * w
    W4 = W2 * W2
    ones = np.ones(S)

    # lhsT coefficient blocks [8, 5, S] (block, K-row, y)
    lcoef = np.zeros((8, 5, S))
    # gn = 1 + b0*w*zs + b1*W2*zs^2          (rhs rows: 1, zs, zs2, zs4, fs)
    lcoef[0] = [ones, b[0] * w, b[1] * W2, 0 * w, 0 * w]
    # gd = 1 - W4*zs^4
    lcoef[1] = [ones, 0 * w, 0 * w, -W4, 0 * w]
    # fz = 1 + fa1*w*zs + fa2*W2*zs^2 + fa4*W4*zs^4
    lcoef[2] = [ones, fa1 * w, fa2 * W2, fa4 * W4, 0 * w]
    # t1 = fz - fs*W4, cancellation-free form (t1 -> 0 as y -> 0):
    #    = fa1*(w-1)*zs + fa2*(W2-1)*zs^2 + fa4*(W4-1)*zs^4 + (1-W4)*fs
    lcoef[3] = [0 * w, fa1 * (w - 1), fa2 * (W2 - 1), fa4 * (W4 - 1), 1.0 - W4]
    # Vd grids: zd = 1 + e*u, e = y2, u = zs-1   (rhs rows: 1, u, u2, u3, u4)
    e = y2
    # zd = 1 + e*u directly (the 1 - zd^4 binomial form suffers a
    # catastrophic cancellation at small zd under the PE's f32r rounding)
    lcoef[4] = [ones, e, 0 * e, 0 * e, 0 * e]
    # fzd = f(zd) = sum_j gam_j e^j u^j ; gam_0 = f(1) = 0 analytically
    g1 = fa1 + 2 * fa2 + 4 * fa4
    g2 = fa2 + 6 * fa4
    g3 = 4 * fa4
    g4 = fa4
    lcoef[5] = [0 * e, g1 * e, g2 * e**2, g3 * e**3, g4 * e**4]
    # gnd = 1 + b0*zd + b1*zd^2 = d0 + d1 e u + d2 e^2 u^2
    d0 = 1.0 + b[0] + b[1]
    d1 = b[0] + 2 * b[1]
    d2 = b[1]
    lcoef[6] = [d0 * ones, d1 * e, d2 * e**2, 0 * e, 0 * e]
    # mu = W4 * fs (exact, multiplicative; used for the cancellation-free
    # Vc term  fz - sqrt(t1*fz) = mu / (1 + sqrt(t1/fz)))
    lcoef[7] = [0 * w, 0 * w, 0 * w, 0 * w, W4]
    # QG = gn*gd expanded over [1, zs, zs2, zs4, zs5, zs6] (rhs block 2 of
    # zrows): rank-6, no harmful cancellation (gd bounded away from 0)
    g1 = b[0] * w
    g2 = b[1] * W2
    qg = np.zeros((1, 6, S))
    qg[0, 0] = ones
    qg[0, 1] = g1
    qg[0, 2] = g2
    qg[0, 3] = -W4
    qg[0, 4] = -g1 * W4
    qg[0, 5] = -g2 * W4
    # -> [5, 7*NT*P]: K-rows on partitions (base 0); block-major, then
    # y-tile, then within-tile index along the free dim, so each matmul's
    # lhsT is lcoef[:, (b*NT+t)*P : (b*NT+t+1)*P]
    lcoef_t = (
        lcoef.reshape(8, 5, NT, P).transpose(1, 0, 2, 3).reshape(5, 8 * NT * P)
        .astype(np.float32).copy()
    )
    # QG block packed separately: [6, NT*P]
    qg_t = (
        qg.reshape(1, 6, NT, P).transpose(1, 0, 2, 3).reshape(6, NT * P)
        .astype(np.float32).copy()
    )

    uL = _trapz_weights(y, append_one=True)
    uD = _trapz_weights(y2, append_one=False)
    # [P, NT*3 + 4]: tile t's three weight columns at cols 3t..3t+2, then
    # two 2-column partition-selector blocks for the tail (L/Vc gather and
    # a zero/Vd gather)
    rw = np.stack([uL * y * W2, uL * y / W2, uD], axis=1)  # [S, 3]
    rwts = np.zeros((P, NT * 3 + 4), np.float32)
    rwts[:, 0:NT * 3] = (
        rw.reshape(NT, P, 3).transpose(1, 0, 2).reshape(P, NT * 3)
    )
    rwts[0, NT * 3 + 0] = 1.0     # Sel1 col0 <- acc row 0  (L)
    rwts[32, NT * 3 + 1] = 1.0    # Sel1 col1 <- acc row 32 (Vc)
    rwts[64, NT * 3 + 3] = 1.0    # Sel2 col1 <- acc row 64 (Vd); col0 = 0

    # per-core zs-derived rows
    zrows_all = []
    srows_all = []
    for c in range(NCORES):
        z = zs[c * BPC:(c + 1) * BPC]
        z2 = z * z
        z4 = z2 * z2
        fs = 1.0 + fa1 * z + fa2 * z2 + fa4 * z4
        u = z - 1.0
        z5 = z4 * z
        z6 = z5 * z
        zrows = np.stack(
            [np.ones(BPC), z, z2, z4, fs, np.ones(BPC), u, u * u, u**3, u**4,
             np.ones(BPC), z, z2, z4, z5, z6]
        ).astype(np.float32)
        scaleL = 4.0 / math.pi * z * np.sqrt(fs)
        sA = math.exp(lc) * 4.0 * math.pi / z
        sB = -math.exp(lc) * 2.0 * math.pi * (1.0 - z)
        srows = np.zeros((2, 3 * BPC))
        srows[0, 0:BPC] = scaleL
        srows[1, 0:BPC] = sA
        srows[1, BPC:2 * BPC] = sB
        srows[1, 2 * BPC:3 * BPC] = sh
        zrows_all.append(zrows)
        srows_all.append(srows.astype(np.float32))
    return lcoef_t, qg_t, rwts, zrows_all, srows_all



def _patch_tile_drain():
    """Walrus rejects instructions with >4 sync waits; Tile's kernel-tail
    drain waits on every active processor at once. Split it into one drain
    per processor (SP-engine drains are ~12 ns each)."""
    import re as _re
    import concourse.tile as tile_mod
    import bass_rust
    from bass_rust import ScopedClock

    if getattr(tile_mod.TileContext, "_drain_patched", False):
        return

    def _patched(self, tick_clock, wait_clock):
        gc = tick_clock.global_clock
        ticks = [int(x) for x in _re.findall(r"\d+", repr(gc))]
        for i in [i for i, t in enumerate(ticks) if t > 0]:
            sub = bass_rust.VectorClock()
            sub.require_at_least(i, ticks[i])
            d = self.nc.sync.drain()
            wait_clock.add_sem_waits(d.ins, ScopedClock({None: sub}))
        self.nc.all_engine_barrier()
        popped = self.nc._tile_sem_poison_stack.pop()
        assert popped is self._sem_poison
        self.nc.clear_and_free_semaphores(list(self.sems.allocated().values()))
        self.nc.all_engine_barrier()

    tile_mod.TileContext._drain_and_barrier = _patched
    tile_mod.TileContext._drain_patched = True


def _prune_redundant_waits(nc):
    """Tile emits per-instruction sem waits that are not transitively minimal
    (syncing on engine X does not teach it what X itself had waited on), but
    every TPB instruction has exactly ONE sync-wait slot. Run a vector-clock
    closure over the scheduled program, drop every wait already implied by
    the instruction's processor, and hoist any excess waits onto earlier
    same-processor instructions with a free slot (cycle-checked)."""
    insts = []
    for blk in nc.m.functions[0].blocks:
        insts.extend(blk.instructions)

    # barrier gather/release semaphores are not monotonic (sem-sub in the
    # butterfly); never prune or reason transitively through them. The
    # end-of-kernel range-clear also resets engine/DMA sems but only after
    # every body wait has fired, so those can stay prunable.
    nonmono = set()
    for inst in insts:
        si = inst.sync_info
        if si is None:
            continue
        for u in si.on_update or []:
            nm = getattr(u, "ant_name", "") or ""
            if getattr(u, "sync_type", "") == "semaphore" and \
                    getattr(u, "update_mode", "") != "sem-inc" and \
                    "barrier" in nm:
                nonmono.add(u.id)
        for w in si.on_wait or []:
            nm = getattr(w, "ant_name", "") or ""
            if "barrier" in nm:
                nonmono.add(w.id)

    V = {}          # processor key -> {sem_id: observed value}
    snap = {}       # sem_id -> {value: dict snapshot}
    cnt = {}        # sem_id -> current value
    own_sem = {}    # processor key -> its own sem id
    # per processor: list of (sync_info, own_tick_at_emit) with a free slot
    free_slots = {}

    def proc_key(inst):
        si = inst.sync_info
        if si is not None:
            for u in si.on_update or []:
                nm = getattr(u, "ant_name", "") or ""
                if nm.startswith("DMA"):
                    return nm
        return str(inst.engine)

    def dep_state(sem, val):
        snaps = snap.get(sem)
        if not snaps:
            return None
        keys = [k for k in snaps if k >= val]
        if not keys:
            return None
        return snaps[min(keys)]

    def merge_from(state, sem, val):
        state[sem] = max(state.get(sem, 0), val)
        ds = dep_state(sem, val)
        if ds:
            for s2, v2 in ds.items():
                if state.get(s2, 0) < v2:
                    state[s2] = v2

    n_dropped = n_hoisted = n_left = 0
    for inst in insts:
        si = inst.sync_info
        pk = proc_key(inst)
        state = V.setdefault(pk, {})
        my_sem = own_sem.get(pk)
        if si is not None and si.on_wait:
            kept = []
            movable = []
            for w in si.on_wait:
                if getattr(w, "sync_type", "") != "semaphore" or \
                        getattr(w, "wait_mode", "") != "sem-ge-imm" or \
                        w.id in nonmono:
                    kept.append(w)
                    continue
                sem, val = w.id, w.wait_value
                if state.get(sem, 0) >= val:
                    n_dropped += 1
                else:
                    movable.append(w)
                merge_from(state, sem, val)
            # hoist all but one movable wait onto earlier free slots
            while len(kept) + len(movable) > 1 and movable:
                w = movable.pop(0)
                placed = False
                for tsi, ttick in reversed(free_slots.get(pk, [])):
                    ds = dep_state(w.id, w.wait_value) or {}
                    # the target's own completion is tick `ttick`; the
                    # producer may only depend on strictly earlier ticks
                    if my_sem is not None and ds.get(my_sem, 0) >= ttick:
                        continue  # would deadlock
                    if not ds:
                        continue  # unknown producer: don't risk it
                    tsi.on_wait = [w]
                    free_slots[pk].remove((tsi, ttick))
                    placed = True
                    n_hoisted += 1
                    break
                if not placed:
                    kept.append(w)
                    n_left += 1
            kept.extend(movable)
            if len(kept) != len(si.on_wait):
                si.on_wait = kept
        if si is not None:
            for u in si.on_update or []:
                if getattr(u, "sync_type", "") != "semaphore":
                    continue
                sem = u.id
                if getattr(u, "update_mode", "") != "sem-inc" or sem in nonmono:
                    continue
                uv = getattr(u, "update_value", 1) or 1
                cnt[sem] = cnt.get(sem, 0) + uv
                if not pk.startswith("DMA"):
                    own_sem.setdefault(pk, sem)
                here = dict(state)
                here[sem] = cnt[sem]
                snap.setdefault(sem, {})[cnt[sem]] = here
                state[sem] = cnt[sem]
        if (si is not None and not si.on_wait and not pk.startswith("DMA")
                and str(getattr(inst, "opcode", "")) not in ("Matmult",)):
            free_slots.setdefault(pk, []).append(
                (si, cnt.get(own_sem.get(pk, -1), 0)))
    if n_left:
        import logging
        logging.warning("_prune_redundant_waits: %d waits could not be "
                        "hoisted; compile may fail", n_left)
    return n_dropped, n_hoisted, n_left


def _act_raw(nc, mybir, func, out, in_, scale=1.0, bias=0.0):
    eng = nc.scalar
    return eng.add_instruction(mybir.InstActivation(
        name=nc.get_next_instruction_name(), func=func,
        ins=[eng.lower_ap(in_),
             mybir.ImmediateValue(dtype=mybir.dt.float32, value=bias),
             mybir.ImmediateValue(dtype=mybir.dt.float32, value=scale),
             mybir.ImmediateValue(dtype=mybir.dt.float32, value=0.0)],
        outs=[eng.lower_ap(out)]))


def _build_nc():
    import concourse.bass as bass
    import concourse.mybir as mybir
    from concourse.tile import TileContext
    from concourse.bass import _add_dep_helper

    f32 = mybir.dt.float32
    f32r = mybir.dt.float32r
    bf16 = mybir.dt.bfloat16
    AF = mybir.ActivationFunctionType

    _patch_tile_drain()
    nc = bass.Bass()
    zrows_d = nc.declare_dram_parameter("zrows", [16, BPC], f32, isOutput=False)
    lcoef_d = nc.declare_dram_parameter("lcoef", [5, 8 * NT * P], f32, isOutput=False)
    qgc_d = nc.declare_dram_parameter("qgc", [6, NT * P], f32, isOutput=False)
    rwts_d = nc.declare_dram_parameter("rwts", [P, NT * 3 + 4], f32, isOutput=False)
    srows_d = nc.declare_dram_parameter("srows", [2, 3 * BPC], f32, isOutput=False)
    out_d = nc.declare_dram_parameter("out", [2, BPC], f32, isOutput=True)

    # The TPB ISA gives a Matmult exactly ONE sync-wait slot and other
    # engine instructions two, so dependencies are funneled: every tensor a
    # poly matmul touches is released by ACT, every reduce-matmul input is
    # DVE-produced, all constants are DMA'd once up front and their DMA
    # queues "absorbed" into PE/DVE clocks by dummy ops. GPSIMD is unused.
    with TileContext(nc) as tc:
        with (
            tc.tile_pool(name="const", bufs=1) as cp,
            tc.tile_pool(name="io", bufs=2) as iop,
            tc.tile_pool(name="work", bufs=2) as wp,
            tc.tile_pool(name="ps", bufs=3, space="PSUM") as pp,
            tc.tile_pool(name="acc", bufs=1, space="PSUM") as accp,
        ):
            # ---- preamble: all constants in one shot ----
            zrc = cp.tile([5, BPC], f32r)
            nc.sync.dma_start(out=zrc[:], in_=zrows_d[0:5, :].bitcast(f32r))
            zrd = cp.tile([5, BPC], f32r)
            nc.sync.dma_start(out=zrd[:], in_=zrows_d[5:10, :].bitcast(f32r))
            lco = cp.tile([5, 8 * NT * P], f32r)
            halfc = 8 * NT * P // 2
            nc.sync.dma_start(out=lco[:, 0:halfc],
                              in_=lcoef_d[:, 0:halfc].bitcast(f32r))
            nc.sync.dma_start(out=lco[:, halfc:],
                              in_=lcoef_d[:, halfc:].bitcast(f32r))
            rw = cp.tile([P, NT * 3 + 4], f32)
            nc.sync.dma_start(out=rw[:], in_=rwts_d[:])
            sr = cp.tile([2, 3 * BPC], f32)
            nc.sync.dma_start(out=sr[:], in_=srows_d[:])

            # accumulators: matmul outs must sit at partition base 0/32/64
            # (row 0 = L, row 32 = Vc, row 64 = Vd); dummy absorber matmuls
            # also write [0:1, 0:1] and are overwritten by the first real
            # start=True accumulation.
            acc = accp.tile([65, 2 * HALF], f32)

            # absorb each const DMA's HW queue into the PE clock so later
            # matmuls never need a DMA wait
            for nm, ap_ in (("zrc", zrc[:, 0:1]), ("zrd", zrd[:, 0:1]),
                            ("lcoA", lco[:, 0:1]), ("lcoB", lco[:, halfc:halfc + 1])):
                a32 = ap_.bitcast(f32)
                nc.tensor.matmul(acc[0:1, 0:1], a32, a32, start=True, stop=True,
                                 skip_group_check=True)
            nc.tensor.matmul(acc[0:1, 0:1], rw[:, 0:1], rw[:, 0:1],
                             start=True, stop=True, skip_group_check=True)
            # rows other than 0/32/64 are never written by the matmuls but
            # are read by the tail gather; zero the tile (after the absorber
            # dummies so their single wait slot stays free for the DMA sems)
            nc.vector.memset(acc[:], 0.0)
            # absorb the srows DMA queue into the DVE clock for the tail ops
            tinyv = cp.tile([1, 1], f32)
            nc.vector.tensor_copy(out=tinyv[:], in_=sr[0:1, 0:1])
            tinya = cp.tile([1, 1], f32)
            tinyp = cp.tile([1, 1], f32)

            def blk(b, t):
                return lco[:, (b * NT + t) * P:(b * NT + t + 1) * P]


            prev_S3 = None
            prev_QGT = None
            for t in range(NT):
                first = t == 0
                last = t == NT - 1
                if prev_S3 is not None:
                    nc.scalar.copy(out=tinya[:], in_=prev_S3[0:1, 0:1])


                # ---- connected grids (y): gn, gd, fz, t1 via PE ----
                gn_ps = pp.tile([P, BPC], f32, tag="poly", name=f"gn_ps{t}")
                gd_ps = pp.tile([P, BPC], f32, tag="poly", name=f"gd_ps{t}")
                fz_ps = pp.tile([P, BPC], f32, tag="poly", name=f"fz_ps{t}")
                t1_ps = pp.tile([P, BPC], f32, tag="poly", name=f"t1_ps{t}")
                mu_ps = pp.tile([P, BPC], f32, tag="poly", name=f"mu_ps{t}")
                for h in range(2):
                    cs = slice(h * HALF, (h + 1) * HALF)
                    rz = zrc[:, cs]
                    nc.tensor.matmul(gn_ps[:, cs], blk(0, t), rz, start=True, stop=True)
                    nc.tensor.matmul(gd_ps[:, cs], blk(1, t), rz, start=True, stop=True)
                    nc.tensor.matmul(fz_ps[:, cs], blk(2, t), rz, start=True, stop=True)
                    nc.tensor.matmul(t1_ps[:, cs], blk(3, t), rz, start=True, stop=True)
                    nc.tensor.matmul(mu_ps[:, cs], blk(7, t), rz, start=True, stop=True)

                mu_sb = wp.tile([P, BPC], f32, tag="mu_sb", name=f"mu_sb{t}")
                nc.vector.tensor_copy(out=mu_sb[:], in_=mu_ps[:])
                gn_sb = wp.tile([P, BPC], f32, tag="gn_sb", name=f"gn_sb{t}")
                nc.scalar.copy(out=gn_sb[:], in_=gn_ps[:])
                gd_sb = wp.tile([P, BPC], f32, tag="gd_sb", name=f"gd_sb{t}")
                nc.scalar.copy(out=gd_sb[:], in_=gd_ps[:])
                fz_sb = wp.tile([P, BPC], f32, tag="fz_sb", name=f"fz_sb{t}")
                nc.scalar.copy(out=fz_sb[:], in_=fz_ps[:])

                QG = wp.tile([P, BPC], f32, tag="QG", name=f"QG{t}", bufs=1)
                if prev_QGT is not None:
                    qg_abs = nc.gpsimd.tensor_copy(out=tinyp[:], in_=prev_QGT[0:1, 0:1])
                else:
                    qg_abs = None
                qg_i = nc.gpsimd.tensor_mul(QG[:], gn_sb[:], gd_sb[:])
                if qg_abs is not None:
                    _add_dep_helper(qg_i.ins, qg_abs.ins, sync=False,
                                    reason="pool absorber order")
                t1_abs = nc.vector.tensor_copy(out=tinyv[:], in_=t1_ps[0:1, 0:1])
                QGT = wp.tile([P, BPC], f32, tag="QGT", name=f"QGT{t}")
                qgt_i = nc.vector.tensor_mul(QGT[:], QG[:], t1_ps[:])
                _add_dep_helper(qgt_i.ins, t1_abs.ins, sync=False,
                                reason="dve absorber order")
                prev_QGT = QGT
                rQ = wp.tile([P, BPC], f32, tag="rQ", name=f"rQ{t}")
                _act_raw(nc, mybir, AF.Rsqrt, rQ[:], QGT[:])
                Sg = wp.tile([P, BPC], bf16, tag="Sg", name=f"Sg{t}")
                nc.vector.tensor_mul(Sg[:], gn_sb[:], rQ[:])

                X = wp.tile([P, BPC], f32, tag="X", name=f"X{t}", bufs=2)
                nc.vector.tensor_mul(X[:], t1_ps[:], fz_sb[:])
                rX = wp.tile([P, BPC], f32, tag="rX", name=f"rX{t}", bufs=1)
                _act_raw(nc, mybir, AF.Rsqrt, rX[:], X[:])
                st = wp.tile([P, BPC], f32, tag="st", name=f"st{t}", bufs=2)
                nc.vector.tensor_mul(st[:], t1_ps[:], rX[:])
                usq = wp.tile([P, BPC], f32, tag="usq", name=f"usq{t}", bufs=1)
                _act_raw(nc, mybir, AF.Square, usq[:], st[:], scale=1.0, bias=1.0)
                rden = wp.tile([P, BPC], f32, tag="rden", name=f"rden{t}", bufs=1)
                _act_raw(nc, mybir, AF.Rsqrt, rden[:], usq[:])
                # E = mu/(1+sqrt(t1/fz)) with mu = W4*fs exact from the PE
                # (an fz - t1 subtraction would be pure f32r noise at y->1)
                E = wp.tile([P, BPC], bf16, tag="E", name=f"E{t}", bufs=2)
                nc.vector.tensor_mul(E[:], mu_sb[:], rden[:])
                D = wp.tile([P, BPC], bf16, tag="D", name=f"D{t}", bufs=2)
                nc.vector.tensor_mul(D[:], Sg[:], E[:])

                wt_r = iop.tile([P, 3], bf16, tag="wt_r")
                nc.vector.tensor_copy(out=wt_r[:], in_=rw[:, 3 * t:3 * t + 3])

                for h in range(2):
                    cs = slice(h * HALF, (h + 1) * HALF)
                    nc.tensor.matmul(acc[0:1, cs], wt_r[:, 0:1], Sg[:, cs],
                                     start=first, stop=last, skip_group_check=True)
                    nc.tensor.matmul(acc[32:33, cs], wt_r[:, 1:2], D[:, cs],
                                     start=first, stop=last, skip_group_check=True)

                # ---- disconnected grid (y2): zd, fzd, gnd via PE ----
                zd_ps = pp.tile([P, BPC], f32, tag="poly", name=f"zd_ps{t}")
                fzd_ps = pp.tile([P, BPC], f32, tag="poly", name=f"fzd_ps{t}")
                gnd_ps = pp.tile([P, BPC], f32, tag="poly", name=f"gnd_ps{t}")
                for h in range(2):
                    cs = slice(h * HALF, (h + 1) * HALF)
                    rv = zrd[:, cs]
                    nc.tensor.matmul(zd_ps[:, cs], blk(4, t), rv, start=True, stop=True)
                    nc.tensor.matmul(fzd_ps[:, cs], blk(5, t), rv, start=True, stop=True)
                    nc.tensor.matmul(gnd_ps[:, cs], blk(6, t), rv, start=True, stop=True)

                zd2 = wp.tile([P, BPC], f32, tag="zd2", name=f"zd2{t}", bufs=1)
                nc.scalar.square(out=zd2[:], in_=zd_ps[:])
                zd4 = wp.tile([P, BPC], f32, tag="zd4", name=f"zd4{t}", bufs=1)
                nc.scalar.square(out=zd4[:], in_=zd2[:])
                gnd_sb = wp.tile([P, BPC], f32, tag="gnd_sb", name=f"gnd_sb{t}")
                nc.scalar.copy(out=gnd_sb[:], in_=gnd_ps[:])

                gdd = wp.tile([P, BPC], f32, tag="gdd", name=f"gdd{t}", bufs=1)
                nc.vector.tensor_scalar(out=gdd[:], in0=zd4[:], scalar1=-1.0,
                                        scalar2=1.0, op0=mybir.AluOpType.mult,
                                        op1=mybir.AluOpType.add)
                Bt = wp.tile([P, BPC], f32, tag="Bt", name=f"Bt{t}", bufs=1)
                bt_abs = nc.gpsimd.tensor_copy(out=tinyp[:], in_=gdd[0:1, 0:1])
                bt_i = nc.gpsimd.tensor_mul(Bt[:], gdd[:], zd4[:])
                _add_dep_helper(bt_i.ins, bt_abs.ins, sync=False,
                                reason="pool absorber order")
                Pt = wp.tile([P, BPC], f32, tag="Pt", name=f"Pt{t}")
                nc.vector.tensor_mul(Pt[:], fzd_ps[:], gnd_sb[:])
                PBt = wp.tile([P, BPC], f32, tag="PBt", name=f"PBt{t}", bufs=1)
                pb_abs = nc.gpsimd.tensor_copy(out=tinyp[:], in_=Pt[0:1, 0:1])
                pb_i = nc.gpsimd.tensor_mul(PBt[:], Pt[:], Bt[:])
                _add_dep_helper(pb_i.ins, pb_abs.ins, sync=False,
                                reason="pool absorber order")
                r2 = wp.tile([P, BPC], f32, tag="r2", name=f"r2{t}", bufs=1)
                _act_raw(nc, mybir, AF.Rsqrt, r2[:], PBt[:])
                S3 = wp.tile([P, BPC], bf16, tag="S3", name=f"S3{t}", bufs=1)
                nc.vector.tensor_mul(S3[:], Pt[:], r2[:])
                prev_S3 = S3

                for h in range(2):
                    cs = slice(h * HALF, (h + 1) * HALF)
                    nc.tensor.matmul(acc[64:65, cs], wt_r[:, 2:3], S3[:, cs],
                                     start=first, stop=last, skip_group_check=True)

            # ---- tail: gather accumulator rows to partition base 0 with
            # selector matmuls, scale, and write out ----
            sums_w = cp.tile([65, BPC], f32)
            nc.scalar.copy(out=sums_w[:], in_=acc[:])
            s2_ps = pp.tile([2, BPC], f32, tag="poly", name="s2_ps")
            sc_ps = pp.tile([2, BPC], f32, tag="poly", name="sc_ps")
            for h in range(2):
                cs = slice(h * HALF, (h + 1) * HALF)
                nc.tensor.matmul(s2_ps[:, cs], rw[0:65, NT * 3:NT * 3 + 2],
                                 sums_w[:, cs], start=True, stop=True)
                nc.tensor.matmul(sc_ps[:, cs], rw[0:65, NT * 3 + 2:NT * 3 + 4],
                                 sums_w[:, cs], start=True, stop=True)
            sums2 = cp.tile([2, BPC], f32)
            nc.scalar.copy(out=sums2[:], in_=s2_ps[:])
            scr = cp.tile([2, BPC], f32)
            nc.scalar.copy(out=scr[:], in_=sc_ps[:])
            pr = cp.tile([2, BPC], f32)
            nc.vector.tensor_mul(pr[:], sums2[:], sr[:, 0:BPC])
            nc.vector.tensor_mul(scr[:], scr[:], sr[:, BPC:2 * BPC])
            nc.vector.tensor_add(pr[:], pr[:], scr[:])
            nc.vector.tensor_add(pr[:], pr[:], sr[:, 2 * BPC:3 * BPC])
            nc.sync.dma_start(out=out_d[:], in_=pr[:])

    _prune_redundant_waits(nc)
    return nc


def _get_nc():
    if "nc" not in _COMPILED:
        _COMPILED["nc"] = _build_nc()
    return _COMPILED["nc"]


def kernel(a, b, logcoef, shift, zs, _trace=False):
    from concourse.bass_utils import run_bass_kernel_spmd

    a = np.asarray(a)
    b = np.asarray(b)
    zs = np.asarray(zs)
    assert zs.shape == (B_TOTAL,)

    lcoef_t, qg_t, rwts, zrows_all, srows_all = _build_host_tables(
        a, b, logcoef, shift, zs)

    in_maps = [
        {
            "zrows": zrows_all[c],
            "lcoef": lcoef_t,
            "qgc": qg_t,
            "rwts": rwts,
            "srows": srows_all[c],
        }
        for c in range(NCORES)
    ]

    nc = _get_nc()
    res = run_bass_kernel_spmd(nc, in_maps, core_ids=list(range(NCORES)),
                               trace=_trace)
    out = np.concatenate([res.results[c]["out"] for c in range(NCORES)], axis=1)
    if _trace:
        kernel.last_exec_time_ns = res.exec_time_ns
        kernel.last_profile = res.profile_json
    return out.astype(np.float32)



# revision 26
# speedup vs baseline: 6.0468x; 6.0468x over previous
"""Trainium2 Bass kernel for the AdSBHNet holographic-potential problem.

Key idea: all three integrands are analytic on y in [0,1] (the apparent
sqrt singularities at the endpoints cancel), so a 16-node Gauss-Legendre
rule reproduces the reference's 1000-point trapezoid to ~2.6e-5 relative
(the reference's own discretization error) -- measured in float64 against
the jax reference. That shrinks the quadrature grid 62x vs the trapz
baseline.

Sharding: data-parallel over zs across 8 NeuronCores (1024 each). Per
core the grid is [128 partitions = 8 zs-groups x 16 y-nodes, 128 free =
zs within group]. All polynomial grids (gn, t1, QG=gn*gd, fz, zd^2,
gdd'', Pt') are rank<=6 in (y-coeffs x zs-power-rows) and are built by
TensorEngine matmuls with block-diagonal (per-group) stationaries
straight into PSUM. DVE/ACT/GPSIMD run the short sqrt/reciprocal chain;
ONE matmul with a per-partition quadrature-weight selector reduces all
three integrals for all 1024 zs at once ([8, 384] PSUM out); the tiny
tail applies per-zs scales, the Vc-Vd combine, and the shift.

Numerics: everything f32/f32r. The Vd y-weight mismatch (w/sqrt(y) vs
w*y*W2) is folded into the Pt'/gdd'' stationary coefficients
(Pt' = Pt*ratio, gdd'' = gdd'/ratio, ratio = 1/(y^1.5 W2)), so one
selector weight serves all three chunks. Catastrophic-cancellation-free
forms: t1 rows vanish as y->0; zd^2 = 1+2x+x^2 (x=e*u) has amplification
~180 which is benign at f32r (fp32) precision; 1-zd^4 is expanded as
(1-zs)*y2*(1+zd+zd^2+zd^3) with the exact (1-zs)*y2 factor folded into
weights/scales.
"""

import math
import numpy as np

B_TOTAL = 8192
NCORES = 8
BPC = B_TOTAL // NCORES          # 1024 zs per core
NY = 16                          # Gauss-Legendre nodes
G = 8                            # zs groups per core
JC = BPC // G                    # 128 zs per group (free dim)

_COMPILED = {}


def _build_host_tables(a, b, logcoef, shift, zs):
    """All derived constants in float64, cast to f32 at the end."""
    a = np.asarray(a, np.float64)
    b = np.asarray(b, np.float64)
    lc = float(np.asarray(logcoef).reshape(-1)[0])
    sh = float(np.asarray(shift).reshape(-1)[0])
    zs = np.asarray(zs, np.float64)

    t, wq = np.polynomial.legendre.leggauss(NY)
    y = 0.5 * (t + 1.0)
    wq = 0.5 * wq                         # nodes/weights on [0,1]

    fa1 = 4.0 / 3.0 * a[0]
    fa2 = 2.0 * a[1]
    fa4 = -(1.0 + fa1 + fa2)

    w1 = 1.0 - y * y
    W2 = w1 * w1
    W4 = W2 * W2
    e = y
    ratio = 1.0 / (y ** 1.5 * W2)         # Vd-weight / LVc-weight
    wL = wq * y * W2                      # the single selector weight
    ones = np.ones(NY)

    # ---- connected lhsT coefficient table [7 kinds, NY] ----
    # rhsC kind order: [1, z, z2, z4, z5, z6, fs]
    def ccoef(d):
        out = np.zeros((7, NY))
        for k, v in d.items():
            out[k] = v
        return out

    gn_c = ccoef({0: ones, 1: b[0] * w1, 2: b[1] * W2})
    fz_c = ccoef({0: ones, 1: fa1 * w1, 2: fa2 * W2, 3: fa4 * W4})
    qg_c = ccoef({0: ones, 1: b[0] * w1, 2: b[1] * W2, 3: -W4,
                  4: -b[0] * w1 * W4, 5: -b[1] * W2 * W4})
    t1_c = ccoef({1: fa1 * (w1 - 1), 2: fa2 * (W2 - 1), 3: fa4 * (W4 - 1),
                  6: 1.0 - W4})

    # ---- disconnected lhsT coefficient table [7 kinds, NY] ----
    # rhsD kind order: [1, u, u2, u3, u4, u5, u6]
    g1 = fa1 + 2 * fa2 + 4 * fa4
    g2 = fa2 + 6 * fa4
    g3 = 4 * fa4
    g4 = fa4
    d0 = 1.0 + b[0] + b[1]
    d1 = b[0] + 2 * b[1]
    d2 = b[1]
    q = np.convolve([0.0, g1, g2, g3, g4], [d0, d1, d2])   # fzd*gnd, powers 0..6

    def dcoef(d):
        out = np.zeros((7, NY))
        for k, v in d.items():
            out[k] = v
        return out

    pt_c = dcoef({m: q[m] * e**m * ratio for m in range(1, 7)})
    gd_c = dcoef({0: 4 * ones / ratio, 1: 6 * e / ratio,
                  2: 4 * e**2 / ratio, 3: e**3 / ratio})
    z2_c = dcoef({0: ones, 1: 2 * e, 2: e * e})

    # ---- block-diagonal stationaries (partition p = g*NY + iy) ----
    def blockdiag(coef, nk):
        # coef [nk, NY] -> lhsT [nk*G rows, 128 cols]
        out = np.zeros((nk * G, G * NY))
        for g in range(G):
            for k in range(nk):
                out[k * G + g, g * NY:(g + 1) * NY] = coef[k]
        return out

    # lhsTC [56, 512]: gn | fz | QG | t1 (t1 spans all 56 rows, zeros on
    # the 1/z5/z6 kind rows -- matmul operands must start at partition 0)
    lhsTC = np.zeros((56, 512))
    lhsTC[0:24, 0:128] = blockdiag(gn_c[0:3], 3)
    lhsTC[0:32, 128:256] = blockdiag(fz_c[0:4], 4)
    lhsTC[0:48, 256:384] = blockdiag(qg_c[0:6], 6)
    lhsTC[0:56, 384:512] = blockdiag(t1_c[0:7], 7)
    # lhsTD [56, 384]: Pt'(all 56 rows, kind-1 rows zero) | gdd'' | zd2
    lhsTD = np.zeros((56, 384))
    lhsTD[0:56, 0:128] = blockdiag(pt_c[0:7], 7)
    lhsTD[0:32, 128:256] = blockdiag(gd_c[0:4], 4)
    lhsTD[0:24, 256:384] = blockdiag(z2_c[0:3], 3)

    # all-ones per-group selector [128, 8] (the y-weights live in scl)
    sel = np.zeros((G * NY, G))
    for g in range(G):
        sel[g * NY:(g + 1) * NY, g] = 1.0

    # ---- per-core zs-derived tables ----
    rhsC_all, rhsD_all, scl_all, ct_all = [], [], [], []
    elc = math.exp(lc)
    for c in range(NCORES):
        z = zs[c * BPC:(c + 1) * BPC]
        z2 = z * z
        z4 = z2 * z2
        fs = 1.0 + fa1 * z + fa2 * z2 + fa4 * z4
        u = z - 1.0

        def rows(kinds):
            # kinds: list of [BPC] arrays -> [len*G, JC] with row k*G+g
            out = np.zeros((len(kinds) * G, JC))
            for k, kv in enumerate(kinds):
                out[k * G:(k + 1) * G, :] = kv.reshape(G, JC)
            return out

        rhsC = rows([np.ones(BPC), z, z2, z4, z4 * z, z4 * z2, fs])
        u2 = u * u
        u3 = u2 * u
        rhsD = rows([np.ones(BPC), u, u2, u3, u2 * u2, u2 * u3, u3 * u3])
        # scl[(g,y), c*JC+j] = wL(y) * c_chunk(zs[g,j]); the Vd y-weight
        # ratio is already folded into the Pt'/gdd'' stationaries
        cL = (4.0 * z * np.sqrt(fs) / math.pi).reshape(G, JC)
        cVc = (4.0 * math.pi * fs * elc / z).reshape(G, JC)
        cVd = (-2.0 * math.pi * np.sqrt(1.0 - z) * elc).reshape(G, JC)
        scl = np.zeros((G * NY, 3 * JC))
        for g in range(G):
            p = slice(g * NY, (g + 1) * NY)
            scl[p, 0:JC] = wL[:, None] * cL[g][None, :]
            scl[p, JC:2 * JC] = wL[:, None] * cVc[g][None, :]
            scl[p, 2 * JC:3 * JC] = wL[:, None] * cVd[g][None, :]
        ct = np.full((G, JC), sh)
        rhsC_all.append(rhsC.astype(np.float32))
        rhsD_all.append(rhsD.astype(np.float32))
        scl_all.append(scl.astype(np.float32))
        ct_all.append(ct.astype(np.float32))

    return (lhsTC.astype(np.float32), lhsTD.astype(np.float32),
            sel.astype(np.float32), rhsC_all, rhsD_all, scl_all, ct_all)


def _patch_tile_drain():
    """Walrus rejects instructions with >4 sync waits; Tile's kernel-tail
    drain waits on every active processor at once. Split it into one drain
    per processor (SP-engine drains are ~12 ns each)."""
    import re as _re
    import concourse.tile as tile_mod
    import bass_rust
    from bass_rust import ScopedClock

    if getattr(tile_mod.TileContext, "_drain_patched", False):
        return

    def _patched(self, tick_clock, wait_clock):
        gc = tick_clock.global_clock
        ticks = [int(x) for x in _re.findall(r"\d+", repr(gc))]
        for i in [i for i, t in enumerate(ticks) if t > 0]:
            sub = bass_rust.VectorClock()
            sub.require_at_least(i, ticks[i])
            d = self.nc.sync.drain()
            wait_clock.add_sem_waits(d.ins, ScopedClock({None: sub}))
        self.nc.all_engine_barrier()
        popped = self.nc._tile_sem_poison_stack.pop()
        assert popped is self._sem_poison
        self.nc.clear_and_free_semaphores(list(self.sems.allocated().values()))
        self.nc.all_engine_barrier()

    tile_mod.TileContext._drain_and_barrier = _patched
    tile_mod.TileContext._drain_patched = True


def _prune_redundant_waits(nc):
    """Tile emits per-instruction sem waits that are not transitively minimal
    (syncing on engine X does not teach it what X itself had waited on), but
    every TPB instruction has exactly ONE sync-wait slot. Run a vector-clock
    closure over the scheduled program, drop every wait already implied by
    the instruction's processor, and hoist any excess waits onto earlier
    same-processor instructions with a free slot (cycle-checked)."""
    insts = []
    for blk in nc.m.functions[0].blocks:
        insts.extend(blk.instructions)

    nonmono = set()
    for inst in insts:
        si = inst.sync_info
        if si is None:
            continue
        for u in si.on_update or []:
            nm = getattr(u, "ant_name", "") or ""
            if getattr(u, "sync_type", "") == "semaphore" and \
                    getattr(u, "update_mode", "") != "sem-inc" and \
                    "barrier" in nm:
                nonmono.add(u.id)
        for w in si.on_wait or []:
            nm = getattr(w, "ant_name", "") or ""
            if "barrier" in nm:
                nonmono.add(w.id)

    V = {}
    snap = {}
    cnt = {}
    own_sem = {}
    free_slots = {}

    def proc_key(inst):
        si = inst.sync_info
        if si is not None:
            for u in si.on_update or []:
                nm = getattr(u, "ant_name", "") or ""
                if nm.startswith("DMA"):
                    return nm
        return str(inst.engine)

    def dep_state(sem, val):
        snaps = snap.get(sem)
        if not snaps:
            return None
        keys = [k for k in snaps if k >= val]
        if not keys:
            return None
        return snaps[min(keys)]

    def merge_from(state, sem, val):
        state[sem] = max(state.get(sem, 0), val)
        ds = dep_state(sem, val)
        if ds:
            for s2, v2 in ds.items():
                if state.get(s2, 0) < v2:
                    state[s2] = v2

    n_dropped = n_hoisted = n_left = 0
    for inst in insts:
        si = inst.sync_info
        pk = proc_key(inst)
        state = V.setdefault(pk, {})
        my_sem = own_sem.get(pk)
        if si is not None and si.on_wait:
            kept = []
            movable = []
            for w in si.on_wait:
                if getattr(w, "sync_type", "") != "semaphore" or \
                        getattr(w, "wait_mode", "") != "sem-ge-imm" or \
                        w.id in nonmono:
                    kept.append(w)
                    continue
                sem, val = w.id, w.wait_value
                if state.get(sem, 0) >= val:
                    n_dropped += 1
                else:
                    movable.append(w)
                merge_from(state, sem, val)
            while len(kept) + len(movable) > 1 and movable:
                w = movable.pop(0)
                placed = False
                for tsi, ttick in reversed(free_slots.get(pk, [])):
                    ds = dep_state(w.id, w.wait_value) or {}
                    if my_sem is not None and ds.get(my_sem, 0) >= ttick:
                        continue
                    if not ds:
                        continue
                    tsi.on_wait = [w]
                    free_slots[pk].remove((tsi, ttick))
                    placed = True
                    n_hoisted += 1
                    break
                if not placed:
                    kept.append(w)
                    n_left += 1
            kept.extend(movable)
            if len(kept) != len(si.on_wait):
                si.on_wait = kept
        if si is not None:
            for u in si.on_update or []:
                if getattr(u, "sync_type", "") != "semaphore":
                    continue
                sem = u.id
                if getattr(u, "update_mode", "") != "sem-inc" or sem in nonmono:
                    continue
                uv = getattr(u, "update_value", 1) or 1
                cnt[sem] = cnt.get(sem, 0) + uv
                if not pk.startswith("DMA"):
                    own_sem.setdefault(pk, sem)
                here = dict(state)
                here[sem] = cnt[sem]
                snap.setdefault(sem, {})[cnt[sem]] = here
                state[sem] = cnt[sem]
        if (si is not None and not si.on_wait and not pk.startswith("DMA")
                and str(getattr(inst, "opcode", "")) not in
                ("Matmult", "EventSemaphore", "Drain",
                 "EventSemaphoreRangeClear", "UnconditionalBranch",
                 "CompareBranch", "SetOrderingMode", "Move", "Notify", "Nop")
                and "barrier" not in (inst.name or "")):
            free_slots.setdefault(pk, []).append(
                (si, cnt.get(own_sem.get(pk, -1), 0)))
    if n_left:
        import logging
        logging.warning("_prune_redundant_waits: %d waits could not be "
                        "hoisted; compile may fail", n_left)
    return n_dropped, n_hoisted, n_left


def _act_raw(nc, mybir, func, out, in_, scale=1.0, bias=0.0):
    eng = nc.scalar
    return eng.add_instruction(mybir.InstActivation(
        name=nc.get_next_instruction_name(), func=func,
        ins=[eng.lower_ap(in_),
             mybir.ImmediateValue(dtype=mybir.dt.float32, value=bias),
             mybir.ImmediateValue(dtype=mybir.dt.float32, value=scale),
             mybir.ImmediateValue(dtype=mybir.dt.float32, value=0.0)],
        outs=[eng.lower_ap(out)]))


def _build_nc():
    import concourse.bass as bass
    import concourse.mybir as mybir
    from concourse.tile import TileContext

    f32 = mybir.dt.float32
    f32r = mybir.dt.float32r
    AF = mybir.ActivationFunctionType

    _patch_tile_drain()
    nc = bass.Bass()
    rhsC_d = nc.declare_dram_parameter("rhsC", [56, JC], f32, isOutput=False)
    rhsD_d = nc.declare_dram_parameter("rhsD", [56, JC], f32, isOutput=False)
    lhsTC_d = nc.declare_dram_parameter("lhsTC", [56, 512], f32, isOutput=False)
    lhsTD_d = nc.declare_dram_parameter("lhsTD", [56, 384], f32, isOutput=False)
    sel_d = nc.declare_dram_parameter("sel", [128, G], f32, isOutput=False)
    scl_d = nc.declare_dram_parameter("scl", [128, 3 * JC], f32, isOutput=False)
    ct_d = nc.declare_dram_parameter("ct", [G, JC], f32, isOutput=False)
    out_d = nc.declare_dram_parameter("out", [2, G, JC], f32, isOutput=True)

    with TileContext(nc) as tc:
        with (
            tc.tile_pool(name="const", bufs=1) as cp,
            tc.tile_pool(name="work", bufs=1) as wp,
            tc.tile_pool(name="ps", bufs=1, space="PSUM") as pp,
        ):
            # ---- constants, one DMA each ----
            rc = cp.tile([56, JC], f32)
            nc.sync.dma_start(out=rc[:], in_=rhsC_d[:])
            lcA = cp.tile([56, 512], f32)
            nc.sync.dma_start(out=lcA[:, 0:256],
                              in_=lhsTC_d[:, 0:256])
            nc.sync.dma_start(out=lcA[:, 256:512],
                              in_=lhsTC_d[:, 256:512])
            rd = cp.tile([56, JC], f32)
            nc.sync.dma_start(out=rd[:], in_=rhsD_d[:])
            ld = cp.tile([56, 384], f32)
            nc.sync.dma_start(out=ld[:], in_=lhsTD_d[:])
            sel = cp.tile([128, G], f32)
            nc.sync.dma_start(out=sel[:], in_=sel_d[:])
            scl = cp.tile([128, 3 * JC], f32)
            nc.sync.dma_start(out=scl[:], in_=scl_d[:])
            ct = cp.tile([G, JC], f32)
            nc.sync.dma_start(out=ct[:], in_=ct_d[:])

            dm = pp.tile([1, 2], f32, tag="dummy")
            # absorb DMA queues whose sems would not fit in wait slots:
            # rc + ld + sel into the PE clock.
            for ap_ in (rc[:, 0:1], ld[:, 0:1], sel[:, 0:1]):
                nc.tensor.matmul(dm[0:1, 0:1], ap_[0:1, 0:1], ap_[0:1, 0:1],
                                 start=True, stop=True, skip_group_check=True)

            # ---- polynomial grids via PE (block-diagonal stationaries) ----
            A = pp.tile([128, 256], f32, tag="A")     # [gn | t1]
            B = pp.tile([128, 256], f32, tag="B")     # [QG | fz]
            Z2 = pp.tile([128, JC], f32, tag="Z2")    # zd^2
            C = pp.tile([128, 256], f32, tag="C")     # [gdd'' | Pt']
            Fp = pp.tile([G, 3 * JC], f32, tag="F")   # reduce output

            nc.tensor.matmul(A[:, 0:128], lcA[0:24, 0:128], rc[0:24, :],
                             start=True, stop=True, skip_group_check=True)
            nc.tensor.matmul(A[:, 128:256], lcA[0:56, 384:512], rc[0:56, :],
                             start=True, stop=True, skip_group_check=True)
            nc.tensor.matmul(B[:, 0:128], lcA[0:48, 256:384], rc[0:48, :],
                             start=True, stop=True, skip_group_check=True)
            nc.tensor.matmul(B[:, 128:256], lcA[0:32, 128:256], rc[0:32, :],
                             start=True, stop=True, skip_group_check=True)
            nc.tensor.matmul(Z2[:], ld[0:24, 256:384], rd[0:24, :],
                             start=True, stop=True, skip_group_check=True)
            nc.tensor.matmul(C[:, 0:128], ld[0:32, 128:256], rd[0:32, :],
                             start=True, stop=True, skip_group_check=True)
            nc.tensor.matmul(C[:, 128:256], ld[0:56, 0:128], rd[0:56, :],
                             start=True, stop=True, skip_group_check=True)

            # ---- connected chain ----
            # (walrus: an instruction may read at most ONE PSUM operand, so
            # t1 -- the operand shared by both products -- goes to SBUF)
            T1S = wp.tile([128, JC], f32, tag="T1S")
            nc.scalar.copy(out=T1S[:], in_=A[:, 128:256])
            # absorb the ACT clock into DVE so the next mul keeps one wait
            tva = cp.tile([1, 1], f32)
            nc.vector.tensor_copy(out=tva[:], in_=T1S[0:1, 0:1])
            MW = wp.tile([128, 256], f32, tag="MW")
            nc.vector.tensor_mul(MW[:, 0:128], B[:, 0:128], T1S[:])
            nc.vector.tensor_mul(MW[:, 128:256], T1S[:], B[:, 128:256])
            RQW = wp.tile([128, 256], f32, tag="RQW")
            _act_raw(nc, mybir, AF.Rsqrt, RQW[:], MW[:])
            RRST = wp.tile([128, 3 * JC], f32, tag="RRST")
            ST = wp.tile([128, JC], f32, tag="ST")
            nc.vector.tensor_mul(RRST[:, 0:128], A[:, 0:128], RQW[:, 0:128])
            nc.gpsimd.tensor_mul(ST[:], T1S[:], RQW[:, 128:256])
            RDEN = wp.tile([128, JC], f32, tag="RDEN")
            _act_raw(nc, mybir, AF.Reciprocal, RDEN[:], ST[:],
                     scale=1.0, bias=1.0)
            nc.vector.tensor_mul(RRST[:, 128:256], RRST[:, 0:128], RDEN[:])

            # ---- disconnected chain (gpsimd never touches PSUM) ----
            Z4 = wp.tile([128, JC], f32, tag="Z4")
            _act_raw(nc, mybir, AF.Square, Z4[:], Z2[:])
            PTS = wp.tile([128, JC], f32, tag="PTS")
            nc.scalar.copy(out=PTS[:], in_=C[:, 128:256])
            G1 = wp.tile([128, JC], f32, tag="G1")
            nc.vector.tensor_mul(G1[:], C[:, 0:128], Z4[:])
            # absorb the DVE clock into Pool so PG keeps one wait
            tvc = cp.tile([1, 1], f32)
            nc.gpsimd.tensor_copy(out=tvc[:], in_=G1[0:1, 0:1])
            PG = wp.tile([128, JC], f32, tag="PG")
            nc.gpsimd.tensor_mul(PG[:], PTS[:], G1[:])
            R2 = wp.tile([128, JC], f32, tag="R2")
            _act_raw(nc, mybir, AF.Rsqrt, R2[:], PG[:])
            nc.gpsimd.tensor_mul(RRST[:, 256:384], PTS[:], R2[:])

            # ---- fold quadrature weights + per-zs scales, then ONE
            # reduce matmul: all 3 integrals for all 1024 zs ----
            tvb = cp.tile([1, 1], f32)
            nc.vector.tensor_copy(out=tvb[:], in_=scl[0:1, 0:1])
            RRS = wp.tile([128, 3 * JC], f32, tag="RRS")
            nc.vector.tensor_mul(RRS[:], RRST[:], scl[:])
            nc.tensor.matmul(Fp[:], sel[:], RRS[:],
                             start=True, stop=True, skip_group_check=True)

            # ---- tail: Vc+Vd combine, shift, out ----
            FS = wp.tile([G, 4 * JC], f32, tag="FS")
            nc.scalar.copy(out=FS[:, 0:3 * JC], in_=Fp[:])
            nc.vector.tensor_add(FS[:, 3 * JC:4 * JC], FS[:, JC:2 * JC],
                                 FS[:, 2 * JC:3 * JC])
            nc.vector.tensor_add(FS[:, JC:2 * JC], FS[:, 3 * JC:4 * JC],
                                 ct[:])
            nc.sync.dma_start(out=out_d[0], in_=FS[:, 0:JC])
            nc.sync.dma_start(out=out_d[1], in_=FS[:, JC:2 * JC])

    _prune_redundant_waits(nc)
    return nc


def _get_nc():
    if "nc" not in _COMPILED:
        _COMPILED["nc"] = _build_nc()
    return _COMPILED["nc"]


def kernel(a, b, logcoef, shift, zs, _trace=False):
    from concourse.bass_utils import run_bass_kernel_spmd

    a = np.asarray(a)
    b = np.asarray(b)
    zs = np.asarray(zs)
    assert zs.shape == (B_TOTAL,)

    (lhsTC, lhsTD, sel, rhsC_all, rhsD_all, scl_all, ct_all) = \
        _build_host_tables(a, b, logcoef, shift, zs)

    in_maps = [
        {
            "rhsC": rhsC_all[c],
            "rhsD": rhsD_all[c],
            "lhsTC": lhsTC,
            "lhsTD": lhsTD,
            "sel": sel,
            "scl": scl_all[c],
            "ct": ct_all[c],
        }
        for c in range(NCORES)
    ]

    nc = _get_nc()
    res = run_bass_kernel_spmd(nc, in_maps, core_ids=list(range(NCORES)),
                               trace=_trace)
    out = np.concatenate(
        [res.results[c]["out"].reshape(2, BPC) for c in range(NCORES)], axis=1)
    if _trace:
        kernel.last_exec_time_ns = res.exec_time_ns
        kernel.last_profile = res.profile_json
    return out.astype(np.float32)


# revision 39
# speedup vs baseline: 6.7611x; 1.1181x over previous
"""Trainium2 Bass kernel for the AdSBHNet holographic-potential problem.

Key idea: all three integrands are analytic on y in [0,1] (the apparent
sqrt singularities at the endpoints cancel), so a 16-node Gauss-Legendre
rule reproduces the reference's 1000-point trapezoid to ~2.6e-5 relative
(the reference's own discretization error) -- measured in float64 against
the jax reference. That shrinks the quadrature grid 62x vs the trapz
baseline.

Sharding: data-parallel over zs across 8 NeuronCores (1024 each). Per
core the grid is [128 partitions = 8 zs-groups x 16 y-nodes, 128 free =
zs within group]. Polynomial grids (gn, fz, t1, gdd'', Pt') are built by
fp32 TensorEngine matmuls (full precision via the LOW/HIGH 2-pass) with
block-diagonal per-group stationaries, split into K<=24 sub-matmuls
accumulating in PSUM (the PE quarter-row-group path is ~3x faster than
K>=32). The rank-1 grids gd = 1 - W4(y)*zs^4 and x = y*(zs-1) come from
tensor_scalar ops with per-partition scalar vectors instead of matmuls.
DVE/ACT/GPSIMD run the short sqrt chain; one f32r matmul with an
all-ones per-group selector reduces all three integrals for all 1024 zs
at once (f32r is safe here: the |element|-mass to |V| amplification is
<= 3, so TF32-level element rounding stays ~1.5e-3); the tiny tail does
the Vc+Vd combine and the shift.

Numerics: the Vd y-weight mismatch (w/sqrt(y) vs w*y*W2) is folded into
the Pt'/gdd'' stationary coefficients (Pt' = Pt*ratio, gdd'' =
gdd'/ratio, ratio = 1/(y^1.5 W2)), so one selector weight serves all
three chunks. Cancellation-free forms: t1 rows vanish as y->0; 1-zd^4 =
(1-zs)*y2*(1+zd+zd^2+zd^3) with the exact (1-zs)*y2 factor folded into
weights/scales. Pt = fzd*gnd as a single polynomial has ~45x coefficient
amplification at zd->0.1, which is why the grid matmuls must be true
fp32, not f32r (TF32-ish): f32r grids fail the 2e-2 gate at small zs.
"""

import math
import numpy as np

B_TOTAL = 8192
NCORES = 8
BPC = B_TOTAL // NCORES          # 1024 zs per core
NY = 16                          # Gauss-Legendre nodes
G = 8                            # zs groups per core
JC = BPC // G                    # 128 zs per group (free dim)

# cst88 column layout: rhsC | rhsD | lhsTC (3 grids) | lhsTD (2 grids)
RC0 = 0            # rhsC [48 rows, 128]
RD0 = 128          # rhsD [72 rows, 128]
LC0 = 256          # lhsTC [48 rows, 3*128]  (gn | fz | t1)
LD0 = 640          # lhsTD [72 rows, 2*128]  (gdd'' | Pt')
CW = 896           # cst88 width

# cst128 column layout
SEL0 = 0           # selector [128, 8]
SCL0 = 8           # scl [128, 384]
CT0 = 392          # shift chunk [8, 128] (partitions 0..7)
Z40 = 520          # zs^4 replicated [128, 128]
UR0 = 648          # (zs-1) replicated [128, 128]
W4C = 776          # -W4(y) per-partition column
EC = 777           # y(p) per-partition column
C2W = 784          # cst128 width (pad to a multiple of 16)

_COMPILED = {}
SPLIT_MM = False


def _build_host_tables(a, b, logcoef, shift, zs):
    """All derived constants in float64, cast to f32 at the end."""
    a = np.asarray(a, np.float64)
    b = np.asarray(b, np.float64)
    lc = float(np.asarray(logcoef).reshape(-1)[0])
    sh = float(np.asarray(shift).reshape(-1)[0])
    zs = np.asarray(zs, np.float64)

    t, wq = np.polynomial.legendre.leggauss(NY)
    y = 0.5 * (t + 1.0)
    wq = 0.5 * wq                         # nodes/weights on [0,1]

    fa1 = 4.0 / 3.0 * a[0]
    fa2 = 2.0 * a[1]
    fa4 = -(1.0 + fa1 + fa2)

    w1 = 1.0 - y * y
    W2 = w1 * w1
    W4 = W2 * W2
    e = y
    ratio = 1.0 / (y ** 1.5 * W2)         # Vd-weight / LVc-weight
    wL = wq * y * W2                      # the single selector weight
    ones = np.ones(NY)

    # connected kinds, 32-aligned blocks: rows 0:24 = {1, z, z2},
    # rows 32:48 = {z4, fs}
    # kind indices: 0='1', 1='z', 2='z2' in block0; 4='z4', 5='fs' in
    # block1 (rows 32:40, 40:48)
    gn_c = {0: ones, 1: b[0] * w1, 2: b[1] * W2}
    fz_c = {0: ones, 1: fa1 * w1, 2: fa2 * W2, 4: fa4 * W4}
    t1_c = {1: fa1 * (w1 - 1), 2: fa2 * (W2 - 1), 4: fa4 * (W4 - 1),
            5: 1.0 - W4}

    # disconnected kinds: rows 0:24 = {1, u, u2}, rows 32:56 = {u3,u4,u5},
    # rows 64:72 = {u6}
    g1 = fa1 + 2 * fa2 + 4 * fa4
    g2 = fa2 + 6 * fa4
    g3 = 4 * fa4
    g4 = fa4
    d0 = 1.0 + b[0] + b[1]
    d1 = b[0] + 2 * b[1]
    d2 = b[1]
    q = np.convolve([0.0, g1, g2, g3, g4], [d0, d1, d2])   # fzd*gnd, powers 0..6

    pt_c = {1: q[1] * e * ratio, 2: q[2] * e**2 * ratio,
            3: q[3] * e**3 * ratio, 4: q[4] * e**4 * ratio,
            5: q[5] * e**5 * ratio, 6: q[6] * e**6 * ratio}
    gd_c = {0: 4 * ones / ratio, 1: 6 * e / ratio, 2: 4 * e**2 / ratio,
            3: e**3 / ratio}

    # kind index -> (row offset) maps
    crow = {0: 0, 1: 8, 2: 16, 4: 32, 5: 40}           # connected, 8 rows/kind
    drow = {0: 0, 1: 8, 2: 16, 3: 32, 4: 40, 5: 48, 6: 64}  # disconnected

    def blockdiag(coefs, rowmap, nrows):
        out = np.zeros((nrows, G * NY))
        for k, cy in coefs.items():
            r0 = rowmap[k]
            for g in range(G):
                out[r0 + g, g * NY:(g + 1) * NY] = cy
        return out

    lhsTC = np.zeros((48, 3 * JC))
    lhsTC[:, 0:JC] = blockdiag(gn_c, crow, 48)
    lhsTC[:, JC:2 * JC] = blockdiag(fz_c, crow, 48)
    lhsTC[:, 2 * JC:3 * JC] = blockdiag(t1_c, crow, 48)
    lhsTD = np.zeros((72, 2 * JC))
    lhsTD[:, 0:JC] = blockdiag(gd_c, drow, 72)
    lhsTD[:, JC:2 * JC] = blockdiag(pt_c, drow, 72)

    cst128_shared = np.zeros((G * NY, C2W))
    for g in range(G):
        cst128_shared[g * NY:(g + 1) * NY, SEL0 + g] = 1.0
        cst128_shared[g * NY:(g + 1) * NY, W4C] = -W4
        cst128_shared[g * NY:(g + 1) * NY, EC] = e

    # ---- per-core zs-derived tables ----
    cst88_all, cst128_all = [], []
    elc = math.exp(lc)
    for c in range(NCORES):
        z = zs[c * BPC:(c + 1) * BPC]
        z2 = z * z
        z4 = z2 * z2
        fs = 1.0 + fa1 * z + fa2 * z2 + fa4 * z4
        u = z - 1.0
        u2 = u * u
        u3 = u2 * u

        def rows(kinds, rowmap, nrows):
            out = np.zeros((nrows, JC))
            for k, kv in kinds.items():
                r0 = rowmap[k]
                out[r0:r0 + G, :] = kv.reshape(G, JC)
            return out

        cst88 = np.zeros((88, CW))
        cst88[0:48, RC0:RC0 + JC] = rows(
            {0: np.ones(BPC), 1: z, 2: z2, 4: z4, 5: fs}, crow, 48)
        cst88[0:72, RD0:RD0 + JC] = rows(
            {0: np.ones(BPC), 1: u, 2: u2, 3: u3, 4: u2 * u2, 5: u2 * u3,
             6: u3 * u3}, drow, 72)
        cst88[0:48, LC0:LC0 + 3 * JC] = lhsTC
        cst88[0:72, LD0:LD0 + 2 * JC] = lhsTD

        cst128 = cst128_shared.copy()
        cL = (4.0 * z * np.sqrt(fs) / math.pi).reshape(G, JC)
        cVc = (4.0 * math.pi * fs * elc / z).reshape(G, JC)
        cVd = (-2.0 * math.pi * np.sqrt(1.0 - z) * elc).reshape(G, JC)
        for g in range(G):
            p = slice(g * NY, (g + 1) * NY)
            cst128[p, SCL0 + 0 * JC:SCL0 + 1 * JC] = wL[:, None] * cL[g][None, :]
            cst128[p, SCL0 + 1 * JC:SCL0 + 2 * JC] = wL[:, None] * cVc[g][None, :]
            cst128[p, SCL0 + 2 * JC:SCL0 + 3 * JC] = wL[:, None] * cVd[g][None, :]
            cst128[p, Z40:Z40 + JC] = z4.reshape(G, JC)[g][None, :]
            cst128[p, UR0:UR0 + JC] = u.reshape(G, JC)[g][None, :]
        cst128[0:G, CT0:CT0 + JC] = sh

        cst88_all.append(cst88.astype(np.float32))
        cst128_all.append(cst128.astype(np.float32))

    return cst88_all, cst128_all


def _patch_tile_drain():
    """Walrus rejects instructions with >4 sync waits; Tile's kernel-tail
    drain waits on every active processor at once. Split it into one drain
    per processor (SP-engine drains are ~12 ns each)."""
    import re as _re
    import concourse.tile as tile_mod
    import bass_rust
    from bass_rust import ScopedClock

    if getattr(tile_mod.TileContext, "_drain_patched", False):
        return

    def _patched(self, tick_clock, wait_clock):
        gc = tick_clock.global_clock
        ticks = [int(x) for x in _re.findall(r"\d+", repr(gc))]
        for i in [i for i, t in enumerate(ticks) if t > 0]:
            sub = bass_rust.VectorClock()
            sub.require_at_least(i, ticks[i])
            d = self.nc.sync.drain()
            wait_clock.add_sem_waits(d.ins, ScopedClock({None: sub}))
        self.nc.all_engine_barrier()
        popped = self.nc._tile_sem_poison_stack.pop()
        assert popped is self._sem_poison
        self.nc.clear_and_free_semaphores(list(self.sems.allocated().values()))
        self.nc.all_engine_barrier()

    tile_mod.TileContext._drain_and_barrier = _patched
    tile_mod.TileContext._drain_patched = True


def _prune_redundant_waits(nc):
    """Tile emits per-instruction sem waits that are not transitively minimal
    (syncing on engine X does not teach it what X itself had waited on), but
    every TPB instruction has exactly ONE sync-wait slot. Run a vector-clock
    closure over the scheduled program, drop every wait already implied by
    the instruction's processor, and hoist any excess waits onto earlier
    same-processor instructions with a free slot (cycle-checked)."""
    insts = []
    for blk in nc.m.functions[0].blocks:
        insts.extend(blk.instructions)

    nonmono = set()
    for inst in insts:
        si = inst.sync_info
        if si is None:
            continue
        for u in si.on_update or []:
            nm = getattr(u, "ant_name", "") or ""
            if getattr(u, "sync_type", "") == "semaphore" and \
                    getattr(u, "update_mode", "") != "sem-inc" and \
                    "barrier" in nm:
                nonmono.add(u.id)
        for w in si.on_wait or []:
            nm = getattr(w, "ant_name", "") or ""
            if "barrier" in nm:
                nonmono.add(w.id)

    V = {}
    snap = {}
    cnt = {}
    own_sem = {}
    free_slots = {}

    def proc_key(inst):
        si = inst.sync_info
        if si is not None:
            for u in si.on_update or []:
                nm = getattr(u, "ant_name", "") or ""
                if nm.startswith("DMA"):
                    return nm
        return str(inst.engine)

    def dep_state(sem, val):
        snaps = snap.get(sem)
        if not snaps:
            return None
        keys = [k for k in snaps if k >= val]
        if not keys:
            return None
        return snaps[min(keys)]

    def merge_from(state, sem, val):
        state[sem] = max(state.get(sem, 0), val)
        ds = dep_state(sem, val)
        if ds:
            for s2, v2 in ds.items():
                if state.get(s2, 0) < v2:
                    state[s2] = v2

    n_dropped = n_hoisted = n_left = 0
    for inst in insts:
        si = inst.sync_info
        pk = proc_key(inst)
        state = V.setdefault(pk, {})
        my_sem = own_sem.get(pk)
        if si is not None and si.on_wait:
            kept = []
            movable = []
            dropped_here = set()
            prestate = dict(state)
            for w in si.on_wait:
                if getattr(w, "sync_type", "") != "semaphore" or \
                        getattr(w, "wait_mode", "") != "sem-ge-imm" or \
                        w.id in nonmono:
                    kept.append(w)
                    continue
                sem, val = w.id, w.wait_value
                # droppable if implied by the processor's prior state or by
                # the transitive closure of the other KEPT waits on this inst
                others = dict(prestate)
                for w2 in si.on_wait:
                    if w2 is w or getattr(w2, "sync_type", "") != "semaphore" \
                            or getattr(w2, "wait_mode", "") != "sem-ge-imm" \
                            or w2.id in nonmono or id(w2) in dropped_here:
                        continue
                    merge_from(others, w2.id, w2.wait_value)
                if others.get(sem, 0) >= val:
                    n_dropped += 1
                    dropped_here.add(id(w))
                else:
                    movable.append(w)
                merge_from(state, sem, val)
            while len(kept) + len(movable) > 1 and movable:
                w = movable.pop(0)
                placed = False
                for tsi, ttick in reversed(free_slots.get(pk, [])):
                    ds = dep_state(w.id, w.wait_value) or {}
                    if my_sem is not None and ds.get(my_sem, 0) >= ttick:
                        continue
                    if not ds:
                        continue
                    tsi.on_wait = [w]
                    free_slots[pk].remove((tsi, ttick))
                    placed = True
                    n_hoisted += 1
                    break
                if not placed:
                    kept.append(w)
                    n_left += 1
            kept.extend(movable)
            if len(kept) != len(si.on_wait):
                si.on_wait = kept
        if si is not None:
            for u in si.on_update or []:
                if getattr(u, "sync_type", "") != "semaphore":
                    continue
                sem = u.id
                if getattr(u, "update_mode", "") != "sem-inc" or sem in nonmono:
                    continue
                uv = getattr(u, "update_value", 1) or 1
                cnt[sem] = cnt.get(sem, 0) + uv
                if not pk.startswith("DMA"):
                    own_sem.setdefault(pk, sem)
                here = dict(state)
                here[sem] = cnt[sem]
                snap.setdefault(sem, {})[cnt[sem]] = here
                state[sem] = cnt[sem]
        if (si is not None and not si.on_wait and not pk.startswith("DMA")
                and str(getattr(inst, "opcode", "")) not in
                ("Matmult", "EventSemaphore", "Drain",
                 "EventSemaphoreRangeClear", "UnconditionalBranch",
                 "CompareBranch", "SetOrderingMode", "Move", "Notify", "Nop")
                and "barrier" not in (inst.name or "")):
            free_slots.setdefault(pk, []).append(
                (si, cnt.get(own_sem.get(pk, -1), 0)))
    if n_left:
        import logging
        logging.warning("_prune_redundant_waits: %d waits could not be "
                        "hoisted; compile may fail", n_left)
    return n_dropped, n_hoisted, n_left


def _act_raw(nc, mybir, func, out, in_, scale=1.0, bias=0.0):
    eng = nc.scalar
    return eng.add_instruction(mybir.InstActivation(
        name=nc.get_next_instruction_name(), func=func,
        ins=[eng.lower_ap(in_),
             mybir.ImmediateValue(dtype=mybir.dt.float32, value=bias),
             mybir.ImmediateValue(dtype=mybir.dt.float32, value=scale),
             mybir.ImmediateValue(dtype=mybir.dt.float32, value=0.0)],
        outs=[eng.lower_ap(out)]))


def _build_nc(prune=True):
    import concourse.bass as bass
    import concourse.mybir as mybir
    from concourse.tile import TileContext

    f32 = mybir.dt.float32
    f32r = mybir.dt.float32r
    AF = mybir.ActivationFunctionType
    ALU = mybir.AluOpType

    _patch_tile_drain()
    nc = bass.Bass()
    cst88_d = nc.declare_dram_parameter("cst88", [88, CW], f32, isOutput=False)
    cst128_d = nc.declare_dram_parameter("cst128", [128, C2W], f32,
                                         isOutput=False)
    out_d = nc.declare_dram_parameter("out", [G, 2 * JC], f32, isOutput=True)

    with TileContext(nc) as tc:
        with (
            tc.tile_pool(name="const", bufs=1) as cp,
            tc.tile_pool(name="work", bufs=1) as wp,
            tc.tile_pool(name="ps", bufs=1, space="PSUM") as pp,
        ):
            # ---- constants: exactly two input DMAs ----
            cs = cp.tile([88, CW], f32)
            nc.sync.dma_start(out=cs[:], in_=cst88_d[:])
            ck = cp.tile([128, C2W], f32)
            nc.sync.dma_start(out=ck[:], in_=cst128_d[:])

            def rc(r0, r1):
                return cs[r0:r1, RC0:RC0 + JC]

            def rd(r0, r1):
                return cs[r0:r1, RD0:RD0 + JC]

            def lc(i, r0, r1):
                return cs[r0:r1, LC0 + i * JC:LC0 + (i + 1) * JC]

            def ld(i, r0, r1):
                return cs[r0:r1, LD0 + i * JC:LD0 + (i + 1) * JC]

            # absorb the cst128 DMA into the PE clock (the reduce matmul
            # reads the selector from it and has only one wait slot)
            dm = pp.tile([1, 2], f32, tag="dummy")
            nc.tensor.matmul(dm[0:1, 0:1], ck[0:1, 0:1], ck[0:1, 0:1],
                             start=True, stop=True, skip_group_check=True)

            # ---- polynomial grids via PE, K<=24 sub-matmuls ----
            A = pp.tile([128, 256], f32, tag="A")     # [gn | fz]
            T1 = pp.tile([128, JC], f32, tag="T1")    # t1
            C = pp.tile([128, 256], f32, tag="C")     # [gdd'' | Pt']
            Fp = pp.tile([G, 3 * JC], f32, tag="F")   # reduce output

            MM = dict(skip_group_check=True)
            if SPLIT_MM:
                # t1 first: it gates the whole connected chain
                nc.tensor.matmul(T1[:], lc(2, 0, 24), rc(0, 24),
                                 start=True, stop=False, **MM)
                nc.tensor.matmul(T1[:], lc(2, 32, 48), rc(32, 48),
                                 start=False, stop=True, **MM)
                nc.tensor.matmul(A[:, 0:128], lc(0, 0, 24), rc(0, 24),
                                 start=True, stop=True, **MM)
                nc.tensor.matmul(A[:, 128:256], lc(1, 0, 24), rc(0, 24),
                                 start=True, stop=False, **MM)
                nc.tensor.matmul(A[:, 128:256], lc(1, 32, 40), rc(32, 40),
                                 start=False, stop=True, **MM)
                nc.tensor.matmul(C[:, 0:128], ld(0, 0, 24), rd(0, 24),
                                 start=True, stop=False, **MM)
                nc.tensor.matmul(C[:, 0:128], ld(0, 32, 40), rd(32, 40),
                                 start=False, stop=True, **MM)
                nc.tensor.matmul(C[:, 128:256], ld(1, 0, 24), rd(0, 24),
                                 start=True, stop=False, **MM)
                nc.tensor.matmul(C[:, 128:256], ld(1, 32, 56), rd(32, 56),
                                 start=False, stop=False, **MM)
                nc.tensor.matmul(C[:, 128:256], ld(1, 64, 72), rd(64, 72),
                                 start=False, stop=True, **MM)
            else:
                nc.tensor.matmul(T1[:], lc(2, 0, 48), rc(0, 48),
                                 start=True, stop=True, **MM)
                nc.tensor.matmul(A[:, 0:128], lc(0, 0, 24), rc(0, 24),
                                 start=True, stop=True, **MM)
                nc.tensor.matmul(A[:, 128:256], lc(1, 0, 40), rc(0, 40),
                                 start=True, stop=True, **MM)
                nc.tensor.matmul(C[:, 0:128], ld(0, 0, 40), rd(0, 40),
                                 start=True, stop=True, **MM)
                nc.tensor.matmul(C[:, 128:256], ld(1, 0, 72), rd(0, 72),
                                 start=True, stop=True, **MM)

            # ---- rank-1 grids from tensor_scalar (per-partition scalars) ----
            GTT = wp.tile([128, 256], f32, tag="GTT")   # [gd*t1 | t1]
            T1S = GTT[:, 128:256]
            nc.scalar.copy(out=T1S, in_=T1[:])
            tva = cp.tile([1, 1], f32)
            nc.vector.tensor_copy(out=tva[:], in_=T1S[0:1, 0:1])
            GD = wp.tile([128, JC], f32, tag="GD")      # 1 - W4*zs^4
            nc.vector.tensor_scalar(out=GD[:], in0=ck[:, Z40:Z40 + JC],
                                    scalar1=ck[:, W4C:W4C + 1],
                                    scalar2=1.0, op0=ALU.mult, op1=ALU.add)
            XD = wp.tile([128, JC], f32, tag="XD")      # x = y*(zs-1)
            nc.vector.tensor_scalar(out=XD[:], in0=ck[:, UR0:UR0 + JC],
                                    scalar1=ck[:, EC:EC + 1],
                                    scalar2=None, op0=ALU.mult)
            # the f32r reduce needs an f32r-declared stationary
            selr = wp.tile([128, G], f32r, tag="selr")
            nc.vector.tensor_copy(out=selr[:], in_=ck[:, SEL0:SEL0 + G])
            # absorb the DVE clock into Pool so the gd*t1 mul keeps one wait
            tvd = cp.tile([1, 1], f32)
            nc.gpsimd.tensor_copy(out=tvd[:], in_=GD[0:1, 0:1])
            nc.gpsimd.tensor_mul(GTT[:, 0:128], GD[:], T1S)   # gd*t1

            # ---- connected chain ----
            tvp = cp.tile([1, 1], f32)
            nc.vector.tensor_copy(out=tvp[:], in_=GTT[0:1, 0:1])
            MW = wp.tile([128, 256], f32, tag="MW")     # [gn*gd*t1 | t1*fz]
            nc.vector.tensor_mul(MW[:], GTT[:], A[:])
            RQW = wp.tile([128, 256], f32, tag="RQW")
            _act_raw(nc, mybir, AF.Rsqrt, RQW[:], MW[:])
            RRST = wp.tile([128, 3 * JC], f32, tag="RRST")
            ST = wp.tile([128, JC], f32, tag="ST")
            nc.vector.tensor_mul(RRST[:, 0:128], A[:, 0:128], RQW[:, 0:128])
            nc.gpsimd.tensor_mul(ST[:], T1S, RQW[:, 128:256])
            USQ = wp.tile([128, JC], f32, tag="USQ")
            _act_raw(nc, mybir, AF.Square, USQ[:], ST[:], scale=1.0, bias=1.0)
            RDEN = wp.tile([128, JC], f32, tag="RDEN")
            _act_raw(nc, mybir, AF.Rsqrt, RDEN[:], USQ[:])
            nc.vector.tensor_mul(RRST[:, 128:256], RRST[:, 0:128], RDEN[:])

            # ---- disconnected chain (gpsimd never touches PSUM) ----
            Z2S = wp.tile([128, JC], f32, tag="Z2S")
            _act_raw(nc, mybir, AF.Square, Z2S[:], XD[:], scale=1.0, bias=1.0)
            Z4S = wp.tile([128, JC], f32, tag="Z4S")
            _act_raw(nc, mybir, AF.Square, Z4S[:], Z2S[:])
            PTS = wp.tile([128, JC], f32, tag="PTS")
            nc.scalar.copy(out=PTS[:], in_=C[:, 128:256])
            G1 = wp.tile([128, JC], f32, tag="G1")
            nc.vector.tensor_mul(G1[:], C[:, 0:128], Z4S[:])
            tvc = cp.tile([1, 1], f32)
            nc.gpsimd.tensor_copy(out=tvc[:], in_=G1[0:1, 0:1])
            PG = wp.tile([128, JC], f32, tag="PG")
            nc.gpsimd.tensor_mul(PG[:], PTS[:], G1[:])
            R2 = wp.tile([128, JC], f32, tag="R2")
            _act_raw(nc, mybir, AF.Rsqrt, R2[:], PG[:])
            nc.gpsimd.tensor_mul(RRST[:, 256:384], PTS[:], R2[:])

            # ---- fold weights+scales, then ONE f32r reduce matmul ----
            tvb = cp.tile([1, 1], f32)
            nc.vector.tensor_copy(out=tvb[:], in_=ck[0:1, SCL0:SCL0 + 1])
            RRS = wp.tile([128, 3 * JC], f32r, tag="RRS")
            nc.vector.tensor_mul(RRS[:], RRST[:], ck[:, SCL0:SCL0 + 3 * JC])
            nc.tensor.matmul(Fp[:], selr[:], RRS[:],
                             start=True, stop=True, **MM)

            # ---- tail: Vc+Vd combine, shift, one out DMA ----
            FS = wp.tile([G, 4 * JC], f32, tag="FS")
            nc.scalar.copy(out=FS[:, 0:3 * JC], in_=Fp[:])
            nc.vector.tensor_add(FS[:, 3 * JC:4 * JC], FS[:, JC:2 * JC],
                                 FS[:, 2 * JC:3 * JC])
            nc.vector.tensor_add(FS[:, JC:2 * JC], FS[:, 3 * JC:4 * JC],
                                 ck[0:G, CT0:CT0 + JC])
            nc.sync.dma_start(out=out_d[:], in_=FS[:, 0:2 * JC])

    if prune:
        _prune_redundant_waits(nc)
    return nc


def _get_nc():
    if "nc" not in _COMPILED:
        _COMPILED["nc"] = _build_nc()
    return _COMPILED["nc"]


def kernel(a, b, logcoef, shift, zs, _trace=False):
    from concourse.bass_utils import run_bass_kernel_spmd

    a = np.asarray(a)
    b = np.asarray(b)
    zs = np.asarray(zs)
    assert zs.shape == (B_TOTAL,)

    cst88_all, cst128_all = _build_host_tables(a, b, logcoef, shift, zs)

    in_maps = [
        {"cst88": cst88_all[c], "cst128": cst128_all[c]}
        for c in range(NCORES)
    ]

    nc = _get_nc()
    res = run_bass_kernel_spmd(nc, in_maps, core_ids=list(range(NCORES)),
                               trace=_trace)
    # out [G, 2*JC]: cols 0:128 = L, 128:256 = V, per group g
    outs = []
    for c in range(NCORES):
        o = res.results[c]["out"]
        outs.append(np.stack([o[:, 0:JC].reshape(BPC),
                              o[:, JC:2 * JC].reshape(BPC)]))
    out = np.concatenate(outs, axis=1)
    if _trace:
        kernel.last_exec_time_ns = res.exec_time_ns
        kernel.last_profile = res.profile_json
    return out.astype(np.float32)


# revision 41
# speedup vs baseline: 7.0044x; 1.0360x over previous
"""Trainium2 Bass kernel for the AdSBHNet holographic-potential problem.

Key idea: all three integrands are analytic on y in [0,1] (the apparent
sqrt singularities at the endpoints cancel), so a 16-node Gauss-Legendre
rule reproduces the reference's 1000-point trapezoid to ~2.6e-5 relative
(the reference's own discretization error) -- measured in float64 against
the jax reference. That shrinks the quadrature grid 62x vs the trapz
baseline.

Sharding: data-parallel over zs across 8 NeuronCores (1024 each). Per
core the grid is [128 partitions = 8 zs-groups x 16 y-nodes, 128 free =
zs within group]. Polynomial grids (gn, fz, t1, gdd'', Pt') are built by
fp32 TensorEngine matmuls (full precision via the LOW/HIGH 2-pass) with
block-diagonal per-group stationaries, split into K<=24 sub-matmuls
accumulating in PSUM (the PE quarter-row-group path is ~3x faster than
K>=32). The rank-1 grids gd = 1 - W4(y)*zs^4 and x = y*(zs-1) come from
tensor_scalar ops with per-partition scalar vectors instead of matmuls.
DVE/ACT/GPSIMD run the short sqrt chain; one f32r matmul with an
all-ones per-group selector reduces all three integrals for all 1024 zs
at once (f32r is safe here: the |element|-mass to |V| amplification is
<= 3, so TF32-level element rounding stays ~1.5e-3); the tiny tail does
the Vc+Vd combine and the shift.

Numerics: the Vd y-weight mismatch (w/sqrt(y) vs w*y*W2) is folded into
the Pt'/gdd'' stationary coefficients (Pt' = Pt*ratio, gdd'' =
gdd'/ratio, ratio = 1/(y^1.5 W2)), so one selector weight serves all
three chunks. Cancellation-free forms: t1 rows vanish as y->0; 1-zd^4 =
(1-zs)*y2*(1+zd+zd^2+zd^3) with the exact (1-zs)*y2 factor folded into
weights/scales. Pt = fzd*gnd as a single polynomial has ~45x coefficient
amplification at zd->0.1, which is why the grid matmuls must be true
fp32, not f32r (TF32-ish): f32r grids fail the 2e-2 gate at small zs.
"""

import math
import numpy as np

B_TOTAL = 8192
NCORES = 8
BPC = B_TOTAL // NCORES          # 1024 zs per core
NY = 16                          # Gauss-Legendre nodes
G = 8                            # zs groups per core
JC = BPC // G                    # 128 zs per group (free dim)

# cst88 column layout (connected data first so DMA chunk 1 = cols 0:512
# unblocks the PE immediately): rhsC | lhsTC | rhsD | lhsTD
RC0 = 0            # rhsC [48 rows, 128]
LC0 = 128          # lhsTC [48 rows, 3*128]  (gn | fz | t1)
RD0 = 512          # rhsD [72 rows, 128]
LD0 = 640          # lhsTD [72 rows, 2*128]  (gdd'' | Pt')
CW = 896           # cst88 width

# cst128 column layout (early-needed data in chunk A = cols 0:272)
Z40 = 0            # zs^4 replicated [128, 128]
UR0 = 128          # (zs-1) replicated [128, 128]
W4C = 256          # -W4(y) per-partition column
EC = 257           # y(p) per-partition column
SEL0 = 264         # selector [128, 8]
SCL0 = 272         # scl [128, 384]
CT0 = 656          # shift chunk [8, 128] (partitions 0..7)
C2W = 784          # cst128 width

_COMPILED = {}
SPLIT_MM = False


def _build_host_tables(a, b, logcoef, shift, zs):
    """All derived constants in float64, cast to f32 at the end."""
    a = np.asarray(a, np.float64)
    b = np.asarray(b, np.float64)
    lc = float(np.asarray(logcoef).reshape(-1)[0])
    sh = float(np.asarray(shift).reshape(-1)[0])
    zs = np.asarray(zs, np.float64)

    t, wq = np.polynomial.legendre.leggauss(NY)
    y = 0.5 * (t + 1.0)
    wq = 0.5 * wq                         # nodes/weights on [0,1]

    fa1 = 4.0 / 3.0 * a[0]
    fa2 = 2.0 * a[1]
    fa4 = -(1.0 + fa1 + fa2)

    w1 = 1.0 - y * y
    W2 = w1 * w1
    W4 = W2 * W2
    e = y
    ratio = 1.0 / (y ** 1.5 * W2)         # Vd-weight / LVc-weight
    wL = wq * y * W2                      # the single selector weight
    ones = np.ones(NY)

    # connected kinds, 32-aligned blocks: rows 0:24 = {1, z, z2},
    # rows 32:48 = {z4, fs}
    # kind indices: 0='1', 1='z', 2='z2' in block0; 4='z4', 5='fs' in
    # block1 (rows 32:40, 40:48)
    gn_c = {0: ones, 1: b[0] * w1, 2: b[1] * W2}
    fz_c = {0: ones, 1: fa1 * w1, 2: fa2 * W2, 4: fa4 * W4}
    t1_c = {1: fa1 * (w1 - 1), 2: fa2 * (W2 - 1), 4: fa4 * (W4 - 1),
            5: 1.0 - W4}

    # disconnected kinds: rows 0:24 = {1, u, u2}, rows 32:56 = {u3,u4,u5},
    # rows 64:72 = {u6}
    g1 = fa1 + 2 * fa2 + 4 * fa4
    g2 = fa2 + 6 * fa4
    g3 = 4 * fa4
    g4 = fa4
    d0 = 1.0 + b[0] + b[1]
    d1 = b[0] + 2 * b[1]
    d2 = b[1]
    q = np.convolve([0.0, g1, g2, g3, g4], [d0, d1, d2])   # fzd*gnd, powers 0..6

    pt_c = {1: q[1] * e * ratio, 2: q[2] * e**2 * ratio,
            3: q[3] * e**3 * ratio, 4: q[4] * e**4 * ratio,
            5: q[5] * e**5 * ratio, 6: q[6] * e**6 * ratio}
    gd_c = {0: 4 * ones / ratio, 1: 6 * e / ratio, 2: 4 * e**2 / ratio,
            3: e**3 / ratio}

    # kind index -> (row offset) maps
    crow = {0: 0, 1: 8, 2: 16, 4: 32, 5: 40}           # connected, 8 rows/kind
    drow = {0: 0, 1: 8, 2: 16, 3: 32, 4: 40, 5: 48, 6: 64}  # disconnected

    def blockdiag(coefs, rowmap, nrows):
        out = np.zeros((nrows, G * NY))
        for k, cy in coefs.items():
            r0 = rowmap[k]
            for g in range(G):
                out[r0 + g, g * NY:(g + 1) * NY] = cy
        return out

    lhsTC = np.zeros((48, 3 * JC))
    lhsTC[:, 0:JC] = blockdiag(gn_c, crow, 48)
    lhsTC[:, JC:2 * JC] = blockdiag(fz_c, crow, 48)
    lhsTC[:, 2 * JC:3 * JC] = blockdiag(t1_c, crow, 48)
    lhsTD = np.zeros((72, 2 * JC))
    lhsTD[:, 0:JC] = blockdiag(gd_c, drow, 72)
    lhsTD[:, JC:2 * JC] = blockdiag(pt_c, drow, 72)

    cst128_shared = np.zeros((G * NY, C2W))
    for g in range(G):
        cst128_shared[g * NY:(g + 1) * NY, SEL0 + g] = 1.0
        cst128_shared[g * NY:(g + 1) * NY, W4C] = -W4
        cst128_shared[g * NY:(g + 1) * NY, EC] = e

    # ---- per-core zs-derived tables ----
    cst88_all, cst128_all = [], []
    elc = math.exp(lc)
    for c in range(NCORES):
        z = zs[c * BPC:(c + 1) * BPC]
        z2 = z * z
        z4 = z2 * z2
        fs = 1.0 + fa1 * z + fa2 * z2 + fa4 * z4
        u = z - 1.0
        u2 = u * u
        u3 = u2 * u

        def rows(kinds, rowmap, nrows):
            out = np.zeros((nrows, JC))
            for k, kv in kinds.items():
                r0 = rowmap[k]
                out[r0:r0 + G, :] = kv.reshape(G, JC)
            return out

        cst88 = np.zeros((88, CW))
        cst88[0:48, RC0:RC0 + JC] = rows(
            {0: np.ones(BPC), 1: z, 2: z2, 4: z4, 5: fs}, crow, 48)
        cst88[0:72, RD0:RD0 + JC] = rows(
            {0: np.ones(BPC), 1: u, 2: u2, 3: u3, 4: u2 * u2, 5: u2 * u3,
             6: u3 * u3}, drow, 72)
        cst88[0:48, LC0:LC0 + 3 * JC] = lhsTC
        cst88[0:72, LD0:LD0 + 2 * JC] = lhsTD

        cst128 = cst128_shared.copy()
        cL = (4.0 * z * np.sqrt(fs) / math.pi).reshape(G, JC)
        cVc = (4.0 * math.pi * fs * elc / z).reshape(G, JC)
        cVd = (-2.0 * math.pi * np.sqrt(1.0 - z) * elc).reshape(G, JC)
        for g in range(G):
            p = slice(g * NY, (g + 1) * NY)
            cst128[p, SCL0 + 0 * JC:SCL0 + 1 * JC] = wL[:, None] * cL[g][None, :]
            cst128[p, SCL0 + 1 * JC:SCL0 + 2 * JC] = wL[:, None] * cVc[g][None, :]
            cst128[p, SCL0 + 2 * JC:SCL0 + 3 * JC] = wL[:, None] * cVd[g][None, :]
            cst128[p, Z40:Z40 + JC] = z4.reshape(G, JC)[g][None, :]
            cst128[p, UR0:UR0 + JC] = u.reshape(G, JC)[g][None, :]
        cst128[0:G, CT0:CT0 + JC] = sh

        cst88_all.append(cst88.astype(np.float32))
        cst128_all.append(cst128.astype(np.float32))

    return cst88_all, cst128_all


def _patch_tile_drain():
    """Walrus rejects instructions with >4 sync waits; Tile's kernel-tail
    drain waits on every active processor at once. Split it into one drain
    per processor (SP-engine drains are ~12 ns each)."""
    import re as _re
    import concourse.tile as tile_mod
    import bass_rust
    from bass_rust import ScopedClock

    if getattr(tile_mod.TileContext, "_drain_patched", False):
        return

    def _patched(self, tick_clock, wait_clock):
        gc = tick_clock.global_clock
        ticks = [int(x) for x in _re.findall(r"\d+", repr(gc))]
        for i in [i for i, t in enumerate(ticks) if t > 0]:
            sub = bass_rust.VectorClock()
            sub.require_at_least(i, ticks[i])
            d = self.nc.sync.drain()
            wait_clock.add_sem_waits(d.ins, ScopedClock({None: sub}))
        self.nc.all_engine_barrier()
        popped = self.nc._tile_sem_poison_stack.pop()
        assert popped is self._sem_poison
        self.nc.clear_and_free_semaphores(list(self.sems.allocated().values()))
        self.nc.all_engine_barrier()

    tile_mod.TileContext._drain_and_barrier = _patched
    tile_mod.TileContext._drain_patched = True


def _prune_redundant_waits(nc):
    """Tile emits per-instruction sem waits that are not transitively minimal
    (syncing on engine X does not teach it what X itself had waited on), but
    every TPB instruction has exactly ONE sync-wait slot. Run a vector-clock
    closure over the scheduled program, drop every wait already implied by
    the instruction's processor, and hoist any excess waits onto earlier
    same-processor instructions with a free slot (cycle-checked)."""
    insts = []
    for blk in nc.m.functions[0].blocks:
        insts.extend(blk.instructions)

    nonmono = set()
    for inst in insts:
        si = inst.sync_info
        if si is None:
            continue
        for u in si.on_update or []:
            nm = getattr(u, "ant_name", "") or ""
            if getattr(u, "sync_type", "") == "semaphore" and \
                    getattr(u, "update_mode", "") != "sem-inc" and \
                    "barrier" in nm:
                nonmono.add(u.id)
        for w in si.on_wait or []:
            nm = getattr(w, "ant_name", "") or ""
            if "barrier" in nm:
                nonmono.add(w.id)

    V = {}
    snap = {}
    cnt = {}
    own_sem = {}
    free_slots = {}

    def proc_key(inst):
        si = inst.sync_info
        if si is not None:
            for u in si.on_update or []:
                nm = getattr(u, "ant_name", "") or ""
                if nm.startswith("DMA"):
                    return nm
        return str(inst.engine)

    def dep_state(sem, val):
        snaps = snap.get(sem)
        if not snaps:
            return None
        keys = [k for k in snaps if k >= val]
        if not keys:
            return None
        return snaps[min(keys)]

    def merge_from(state, sem, val):
        state[sem] = max(state.get(sem, 0), val)
        ds = dep_state(sem, val)
        if ds:
            for s2, v2 in ds.items():
                if state.get(s2, 0) < v2:
                    state[s2] = v2

    n_dropped = n_hoisted = n_left = 0
    for inst in insts:
        si = inst.sync_info
        pk = proc_key(inst)
        state = V.setdefault(pk, {})
        my_sem = own_sem.get(pk)
        if si is not None and si.on_wait:
            kept = []
            movable = []
            dropped_here = set()
            prestate = dict(state)
            for w in si.on_wait:
                if getattr(w, "sync_type", "") != "semaphore" or \
                        getattr(w, "wait_mode", "") != "sem-ge-imm" or \
                        w.id in nonmono:
                    kept.append(w)
                    continue
                sem, val = w.id, w.wait_value
                # droppable if implied by the processor's prior state or by
                # the transitive closure of the other KEPT waits on this inst
                others = dict(prestate)
                for w2 in si.on_wait:
                    if w2 is w or getattr(w2, "sync_type", "") != "semaphore" \
                            or getattr(w2, "wait_mode", "") != "sem-ge-imm" \
                            or w2.id in nonmono or id(w2) in dropped_here:
                        continue
                    merge_from(others, w2.id, w2.wait_value)
                if others.get(sem, 0) >= val:
                    n_dropped += 1
                    dropped_here.add(id(w))
                else:
                    movable.append(w)
                merge_from(state, sem, val)
            while len(kept) + len(movable) > 1 and movable:
                w = movable.pop(0)
                placed = False
                for tsi, ttick in reversed(free_slots.get(pk, [])):
                    ds = dep_state(w.id, w.wait_value) or {}
                    if my_sem is not None and ds.get(my_sem, 0) >= ttick:
                        continue
                    if not ds:
                        continue
                    tsi.on_wait = [w]
                    free_slots[pk].remove((tsi, ttick))
                    placed = True
                    n_hoisted += 1
                    break
                if not placed:
                    kept.append(w)
                    n_left += 1
            kept.extend(movable)
            if len(kept) != len(si.on_wait):
                si.on_wait = kept
        if si is not None:
            for u in si.on_update or []:
                if getattr(u, "sync_type", "") != "semaphore":
                    continue
                sem = u.id
                if getattr(u, "update_mode", "") != "sem-inc" or sem in nonmono:
                    continue
                uv = getattr(u, "update_value", 1) or 1
                cnt[sem] = cnt.get(sem, 0) + uv
                if not pk.startswith("DMA"):
                    own_sem.setdefault(pk, sem)
                here = dict(state)
                here[sem] = cnt[sem]
                snap.setdefault(sem, {})[cnt[sem]] = here
                state[sem] = cnt[sem]
        if (si is not None and not si.on_wait and not pk.startswith("DMA")
                and str(getattr(inst, "opcode", "")) not in
                ("Matmult", "EventSemaphore", "Drain",
                 "EventSemaphoreRangeClear", "UnconditionalBranch",
                 "CompareBranch", "SetOrderingMode", "Move", "Notify", "Nop")
                and "barrier" not in (inst.name or "")):
            free_slots.setdefault(pk, []).append(
                (si, cnt.get(own_sem.get(pk, -1), 0)))
    if n_left:
        import logging
        logging.warning("_prune_redundant_waits: %d waits could not be "
                        "hoisted; compile may fail", n_left)
    return n_dropped, n_hoisted, n_left


def _act_raw(nc, mybir, func, out, in_, scale=1.0, bias=0.0):
    eng = nc.scalar
    return eng.add_instruction(mybir.InstActivation(
        name=nc.get_next_instruction_name(), func=func,
        ins=[eng.lower_ap(in_),
             mybir.ImmediateValue(dtype=mybir.dt.float32, value=bias),
             mybir.ImmediateValue(dtype=mybir.dt.float32, value=scale),
             mybir.ImmediateValue(dtype=mybir.dt.float32, value=0.0)],
        outs=[eng.lower_ap(out)]))


def _build_nc(prune=True):
    import concourse.bass as bass
    import concourse.mybir as mybir
    from concourse.tile import TileContext

    f32 = mybir.dt.float32
    f32r = mybir.dt.float32r
    AF = mybir.ActivationFunctionType
    ALU = mybir.AluOpType

    _patch_tile_drain()
    nc = bass.Bass(enable_partition_id=False)
    cst88_d = nc.declare_dram_parameter("cst88", [88, CW], f32, isOutput=False)
    cst128_d = nc.declare_dram_parameter("cst128", [128, C2W], f32,
                                         isOutput=False)
    out_d = nc.declare_dram_parameter("out", [G, 2 * JC], f32, isOutput=True)

    with TileContext(nc) as tc:
        with (
            tc.tile_pool(name="const", bufs=1) as cp,
            tc.tile_pool(name="work", bufs=1) as wp,
            tc.tile_pool(name="ps", bufs=1, space="PSUM") as pp,
        ):
            # ---- constants: four DMAs on parallel queues, earliest-needed
            # data first ----
            cs = cp.tile([88, CW], f32)
            nc.sync.dma_start(out=cs[:, 0:512], in_=cst88_d[:, 0:512])
            ck = cp.tile([128, C2W], f32)
            nc.sync.dma_start(out=ck[:, 0:SCL0], in_=cst128_d[:, 0:SCL0])
            nc.sync.dma_start(out=cs[:, 512:CW], in_=cst88_d[:, 512:CW])
            nc.sync.dma_start(out=ck[:, SCL0:C2W], in_=cst128_d[:, SCL0:C2W])

            def rc(r0, r1):
                return cs[r0:r1, RC0:RC0 + JC]

            def rd(r0, r1):
                return cs[r0:r1, RD0:RD0 + JC]

            def lc(i, r0, r1):
                return cs[r0:r1, LC0 + i * JC:LC0 + (i + 1) * JC]

            def ld(i, r0, r1):
                return cs[r0:r1, LD0 + i * JC:LD0 + (i + 1) * JC]

            # ---- polynomial grids via PE (fp32 LOW/HIGH, single MM each:
            # fp32 multi-matmul PSUM accumulation hangs the HW) ----
            T1 = pp.tile([128, JC], f32, tag="T1")    # t1
            A = pp.tile([128, 256], f32, tag="A")     # [gn | fz]
            C = pp.tile([128, 256], f32, tag="C")     # [gdd'' | Pt']
            Fp = pp.tile([G, 3 * JC], f32, tag="F")   # reduce output

            MM = dict(skip_group_check=True)
            # t1 first: it gates the whole connected chain
            nc.tensor.matmul(T1[:], lc(2, 0, 48), rc(0, 48),
                             start=True, stop=True, **MM)
            nc.tensor.matmul(A[:, 0:128], lc(0, 0, 24), rc(0, 24),
                             start=True, stop=True, **MM)
            nc.tensor.matmul(A[:, 128:256], lc(1, 0, 40), rc(0, 40),
                             start=True, stop=True, **MM)
            nc.tensor.matmul(C[:, 0:128], ld(0, 0, 40), rd(0, 40),
                             start=True, stop=True, **MM)
            nc.tensor.matmul(C[:, 128:256], ld(1, 0, 72), rd(0, 72),
                             start=True, stop=True, **MM)

            # ---- rank-1 grids from tensor_scalar (per-partition scalars) ----
            GD = wp.tile([128, JC], f32, tag="GD")      # 1 - W4*zs^4
            nc.vector.tensor_scalar(out=GD[:], in0=ck[:, Z40:Z40 + JC],
                                    scalar1=ck[:, W4C:W4C + 1],
                                    scalar2=1.0, op0=ALU.mult, op1=ALU.add)
            XD = wp.tile([128, JC], f32, tag="XD")      # x = y*(zs-1)
            nc.vector.tensor_scalar(out=XD[:], in0=ck[:, UR0:UR0 + JC],
                                    scalar1=ck[:, EC:EC + 1],
                                    scalar2=None, op0=ALU.mult)
            # the f32r reduce needs an f32r-declared stationary
            selr = wp.tile([128, G], f32r, tag="selr")
            nc.vector.tensor_copy(out=selr[:], in_=ck[:, SEL0:SEL0 + G])

            # ---- connected chain.  T1S (SBUF copy of t1, for engines that
            # cannot read PSUM) is produced on ACT off the critical path;
            # gd*t1 reads t1 straight from PSUM on DVE. ----
            T1S = wp.tile([128, JC], f32, tag="T1S")
            nc.scalar.copy(out=T1S[:], in_=T1[:])
            MW = wp.tile([128, 256], f32, tag="MW")     # [gn*gd*t1 | t1*fz]
            GT = wp.tile([128, JC], f32, tag="GT")      # gd*t1
            nc.vector.tensor_mul(GT[:], T1[:], GD[:])
            nc.vector.tensor_mul(MW[:, 0:128], A[:, 0:128], GT[:])
            tva = cp.tile([1, 1], f32)
            nc.vector.tensor_copy(out=tva[:], in_=T1S[0:1, 0:1])
            nc.vector.tensor_mul(MW[:, 128:256], A[:, 128:256], T1S[:])
            RQW = wp.tile([128, 256], f32, tag="RQW")
            _act_raw(nc, mybir, AF.Rsqrt, RQW[:], MW[:])
            RRST = wp.tile([128, 3 * JC], f32, tag="RRST")
            ST = wp.tile([128, JC], f32, tag="ST")
            nc.vector.tensor_mul(RRST[:, 0:128], A[:, 0:128], RQW[:, 0:128])
            nc.gpsimd.tensor_mul(ST[:], T1S[:], RQW[:, 128:256])
            USQ = wp.tile([128, JC], f32, tag="USQ")
            _act_raw(nc, mybir, AF.Square, USQ[:], ST[:], scale=1.0, bias=1.0)
            RDEN = wp.tile([128, JC], f32, tag="RDEN")
            _act_raw(nc, mybir, AF.Rsqrt, RDEN[:], USQ[:])
            nc.vector.tensor_mul(RRST[:, 128:256], RRST[:, 0:128], RDEN[:])

            # ---- disconnected chain (gpsimd never touches PSUM) ----
            Z2S = wp.tile([128, JC], f32, tag="Z2S")
            _act_raw(nc, mybir, AF.Square, Z2S[:], XD[:], scale=1.0, bias=1.0)
            Z4S = wp.tile([128, JC], f32, tag="Z4S")
            _act_raw(nc, mybir, AF.Square, Z4S[:], Z2S[:])
            PTS = wp.tile([128, JC], f32, tag="PTS")
            nc.scalar.copy(out=PTS[:], in_=C[:, 128:256])
            tvz = cp.tile([1, 1], f32)
            nc.vector.tensor_copy(out=tvz[:], in_=Z4S[0:1, 0:1])
            G1 = wp.tile([128, JC], f32, tag="G1")
            nc.vector.tensor_mul(G1[:], C[:, 0:128], Z4S[:])
            tvc = cp.tile([1, 1], f32)
            nc.gpsimd.tensor_copy(out=tvc[:], in_=G1[0:1, 0:1])
            PG = wp.tile([128, JC], f32, tag="PG")
            nc.gpsimd.tensor_mul(PG[:], PTS[:], G1[:])
            R2 = wp.tile([128, JC], f32, tag="R2")
            _act_raw(nc, mybir, AF.Rsqrt, R2[:], PG[:])
            nc.gpsimd.tensor_mul(RRST[:, 256:384], PTS[:], R2[:])

            # ---- fold weights+scales, then ONE f32r reduce matmul ----
            tvb = cp.tile([1, 1], f32)
            nc.vector.tensor_copy(out=tvb[:], in_=ck[0:1, SCL0:SCL0 + 1])
            RRS = wp.tile([128, 3 * JC], f32r, tag="RRS")
            nc.vector.tensor_mul(RRS[:], RRST[:], ck[:, SCL0:SCL0 + 3 * JC])
            nc.tensor.matmul(Fp[:], selr[:], RRS[:],
                             start=True, stop=True, **MM)

            # ---- tail: Vc+Vd combine, shift, one out DMA ----
            FS = wp.tile([G, 4 * JC], f32, tag="FS")
            nc.scalar.copy(out=FS[:, 0:3 * JC], in_=Fp[:])
            nc.vector.tensor_add(FS[:, 3 * JC:4 * JC], FS[:, JC:2 * JC],
                                 FS[:, 2 * JC:3 * JC])
            nc.vector.tensor_add(FS[:, JC:2 * JC], FS[:, 3 * JC:4 * JC],
                                 ck[0:G, CT0:CT0 + JC])
            nc.sync.dma_start(out=out_d[:], in_=FS[:, 0:2 * JC])

    if prune:
        _prune_redundant_waits(nc)
    return nc


def _get_nc():
    if "nc" not in _COMPILED:
        _COMPILED["nc"] = _build_nc()
    return _COMPILED["nc"]


def kernel(a, b, logcoef, shift, zs, _trace=False):
    from concourse.bass_utils import run_bass_kernel_spmd

    a = np.asarray(a)
    b = np.asarray(b)
    zs = np.asarray(zs)
    assert zs.shape == (B_TOTAL,)

    cst88_all, cst128_all = _build_host_tables(a, b, logcoef, shift, zs)

    in_maps = [
        {"cst88": cst88_all[c], "cst128": cst128_all[c]}
        for c in range(NCORES)
    ]

    nc = _get_nc()
    res = run_bass_kernel_spmd(nc, in_maps, core_ids=list(range(NCORES)),
                               trace=_trace)
    # out [G, 2*JC]: cols 0:128 = L, 128:256 = V, per group g
    outs = []
    for c in range(NCORES):
        o = res.results[c]["out"]
        outs.append(np.stack([o[:, 0:JC].reshape(BPC),
                              o[:, JC:2 * JC].reshape(BPC)]))
    out = np.concatenate(outs, axis=1)
    if _trace:
        kernel.last_exec_time_ns = res.exec_time_ns
        kernel.last_profile = res.profile_json
    return out.astype(np.float32)
